# revision 27
# baseline (speedup 1.0000x reference)
"""GCN diag-encoder (2-layer SpMM) on 8 Trainium2 NeuronCores.

Layer 1 (the f16 baseline, unchanged): the sparse adjacency is u8-quantized
per sorted 4-column dst groups (values in [0,247]), streamed once as raw u8
(13.1MB/core), cast on-chip to f16 across the three free engines, and
contracted A-stationary on the PE at f16 rate.  Exact-u8 precision; DMA
(36us) and PE (43us) nearly balanced.

Layer 2 (new): fp8 DoubleRow matmuls at 0.5 cycles/row, still exact-u8.
q = c + r where c = RTNE(q) on the e4m3 grid (one cheap u8->fp8 tensor_copy
from the SBUF-resident q) and r = q - c in [-8,8] (integers, exact in e4m3)
streamed from the host as fp8 into layer 2's otherwise idle DMA window.
The tanh output h is split on-chip into exact fp8 limbs h_hi + h_lo.  Per
k-tile pair, three DoubleRow matmuls with naturally-strided plane pairs:
  (h_hi, h_hi') x (c, c')   + (h_hi, h_hi') x (r, r')   = h_hi * q  exact
  (h_lo, h_lo') x (c, c')                               (drops h_lo*r ~0.07%)
Layer 2's window drops from 43us (f16 PE-bound) to ~35us, balanced between
the r-stream DMA, the DoubleRow PE work, and the c-convert ops.

DMA instruction count is a first-order cost (565ns of SP sequencer
occupancy each), so the r-stream issues one 3D-AP DMA per THREE pair-groups
into a manually-managed ring (one flat tile, slots indexed mod NCR), and
the post-collective x1 loads ride the ACT queue.  The strided DoubleRow
plane reads evade subtile dependency tracking, so L2 matmuls carry explicit
sync deps on their ring producers and producers depend on the matmuls of
the groups that previously owned their slots.

Kept from the baseline: dst-column quant-sorting so the tanh dequant is 3
bank-wide activations, per-bank psum tiles, full-bank psum seeds (a
start=True on a sub-bank range would re-arm the whole 2KB zero-region and
wipe neighbors), rotated src row-blocks (own rank first so L2 starts from
on-chip activations during the AllGather), keep-warm matmuls across the
collective valley, and the high-priority boundary chain.
"""

import numpy as np

N = 10000          # nodes
D = 128            # feature dim
NCORES = 8
S = 1250           # dst nodes per core
SP = 1280          # padded dst per core (10 tiles of 128)
KT = 80            # contraction k-tiles (padded src rows = 10240)
NPAD = KT * 128    # 10240
GSIZES = (2, 2) + (4,) * 19      # L1 k-tiles per cast group (sum = 80)
NCAST = 3          # L1 f16 staging ring depth
NCR = 8            # L2 (c,r) ring slots (1 k-tile pair each)
RSPAN = 2          # pair-groups covered per r-stream DMA
NDUMMY = 6         # PE keep-warm matmuls bridging the AllGather valley
QMAX = 247         # u8 quant ceiling (so RTNE(q) cannot round to fp8 inf)
# L1 psum bank chunks (tanh eviction granularity)
CHUNKS = ((0, 512), (512, 512), (1024, 256))
# L2 psum chunks: DoubleRow moving free dim is 2*cn <= 512
CH2 = tuple((i * 256, 256) for i in range(5))
NPADROW = 113
# c-convert engine spans per k-tile (DVE:ACT:Pool ~ 1.87:1.15:0.71)
CSPANS = (("v", 0, 640), ("a", 640, 1040), ("p", 1040, 1280))


def _rows(k):
    return NPADROW if k % 10 in (8, 9) else 128


def _pair_src(j):
    """Map first k-tile index j=2g of a pair to (L1 group, offset in it)."""
    acc = 0
    for gi, sz in enumerate(GSIZES):
        if j < acc + sz:
            return gi, j - acc
        acc += sz
    raise AssertionError

_PROG_CACHE = {}


def _build_program(nocc=False, gsizes=GSIZES, ncast=NCAST, w1_ones=True,
                   ndummy=NDUMMY):
    import concourse.bacc as bacc
    import concourse.mybir as mybir
    from bass_rust import InstructionNameOrderedSet as _NameSet
    from concourse import tile

    f32 = mybir.dt.float32
    f16 = mybir.dt.float16
    f8 = mybir.dt.float8e4
    u8 = mybir.dt.uint8
    DR = mybir.MatmulPerfMode.DoubleRow
    grps = []
    _k0 = 0
    for _sz in gsizes:
        grps.append((_k0, _k0 + _sz))
        _k0 += _sz
    assert _k0 == KT
    maxg = max(k1 - k0 for k0, k1 in grps)

    nc = bacc.Bacc(
        "TRN2",
        target_bir_lowering=False,
        debug=False,
        enable_asserts=False,
        num_devices=1 if nocc else NCORES,
    )

    a = nc.dram_tensor("a", [KT, 128, SP], u8, kind="ExternalInput").ap()
    ar = nc.dram_tensor("ar", [KT, 128, SP], f8, kind="ExternalInput").ap()
    x0 = nc.dram_tensor("x0", [128, NPAD], f16, kind="ExternalInput").ap()
    csc = nc.dram_tensor("csc", [128, 3], f32, kind="ExternalInput").ap()
    w1c = nc.dram_tensor("w1c", [128, 1], f32, kind="ExternalInput").ap()
    out = nc.dram_tensor("out", [128, SP], f16, kind="ExternalOutput").ap()

    with tile.TileContext(nc) as tc:
        with (
            tc.tile_pool(name="xp", bufs=1) as xp,
            tc.tile_pool(name="a8p", bufs=1) as a8p,
            tc.tile_pool(name="fc", bufs=ncast) as fcp,
            tc.tile_pool(name="th", bufs=2) as thp,
            tc.tile_pool(name="ps", bufs=1, space="PSUM") as ps,
            tc.tile_pool(name="dr", bufs=1, space="DRAM") as drp,
        ):
            # x0 (f16, dead once L1 finishes) shares its slot with x1s
            x0s = xp.tile([128, NPAD], f16, tag="xs", name="x0s")
            x1s = xp.tile([128, 2 * NPAD], f8, tag="xs", name="x1s")
            # manual (c,r) ring: slot g%NCR at offset (g%NCR)*2*2*SP
            crs = xp.tile([128, NCR * 2 * 2 * SP], f8, tag="crs")
            cscs = xp.tile([128, 3], f32, tag="cscs")
            w1s = xp.tile([128, 1], f32, tag="w1s")
            zl = xp.tile([128, 512], f16, tag="zl")
            warm = xp.tile([128, 1], f32, tag="warm")
            agin_sb = xp.tile([128, 2 * SP], f8, tag="agin")  # [hi][lo]
            h16 = xp.tile([128, 2 * SP], f16, tag="h16")  # exact h, kt 60-79
            ob = xp.tile([128, SP], f16, tag="ob")
            nc.vector.memset(zl[:], 0.0)
            nc.scalar.activation(
                warm[:], zl[:, 0:1], mybir.ActivationFunctionType.Tanh
            )

            agin = drp.tile([128, 2 * SP], f8)
            agout = drp.tile([NCORES * 128, 2 * SP], f8, addr_space="Shared")

            a8_tiles = {}

            # ---- layer 1 (A-stationary f16; psum is [dst slot, feat]) ----
            psum1 = []
            for ci, (c0, cn) in enumerate(CHUNKS):
                p1t = ps.tile([128, cn], f32, tag=f"acc1_{ci}", name=f"p1_{ci}")
                psum1.append(p1t)
            for ci, (c0, cn) in enumerate(CHUNKS):
                nc.tensor.matmul(
                    psum1[ci][:, 0:cn], zl[:, 0:128], zl[:, 0:cn],
                    start=True, stop=False,
                )
            xlead = grps[1][1] * 128
            for gi, (k0, k1) in enumerate(grps):
                a8g = a8p.tile([128, (k1 - k0) * SP], u8, tag=f"a8_{gi}")
                a8_tiles[gi] = a8g
                nc.sync.dma_start(
                    a8g[:].rearrange("p (k j) -> p k j", k=k1 - k0),
                    a[k0:k1].rearrange("k p j -> p k j"),
                )
                if gi == 0:
                    nc.sync.dma_start(x0s[:, 0:xlead], x0[:, 0:xlead])
                if gi == 1:
                    nc.scalar.dma_start(cscs[:], csc)
                    if not w1_ones:
                        nc.scalar.dma_start(w1s[:], w1c)
                # x0 tail in 4 coarse spans interleaved with the A stream
                xspans = {2: (xlead, 2048), 5: (2048, 3584),
                          8: (3584, 5632), 11: (5632, 8192),
                          15: (8192, NPAD)}
                if gi in xspans:
                    sp0, sp1 = xspans[gi]
                    nc.sync.dma_start(x0s[:, sp0:sp1], x0[:, sp0:sp1])

            # prefill the L2 (c,r) ring's r-planes while the DMA queue
            # drains the tail of the A stream (before any boundary DMAs)
            pre_rdma = {}
            for g0 in range(0, 2 * RSPAN, RSPAN):
                nkt = 2 * RSPAN
                soff = (g0 % NCR) * 2 * 2 * SP
                rd = nc.sync.dma_start(
                    crs[:, soff:soff + nkt * 2 * SP].rearrange(
                        "p (k two j) -> p k two j", two=2, j=SP
                    )[:, :, 1, :],
                    ar[2 * g0:2 * g0 + nkt].rearrange("k p j -> p k j"),
                )
                pre_rdma[g0] = rd.ins.name

            def cast_group_f16(gi):
                k0, k1 = grps[gi]
                w = (k1 - k0) * SP
                a8g = a8_tiles[gi]
                fb = fcp.tile([128, maxg * SP], f16, tag="fc")
                rates = {"v": 1.87, "a": 1.15, "p": 0.71}
                tot = sum(rates.values())
                c0 = 0
                for e in ("v", "a", "p"):
                    c1 = w if e == "p" else \
                        c0 + int(w * rates[e] / tot) // 16 * 16
                    if e == "v":
                        nc.vector.tensor_copy(fb[:, c0:c1], a8g[:, c0:c1])
                    elif e == "a":
                        s0 = c0
                        while s0 < c1:
                            s1 = min(s0 + 1280, c1)
                            nc.scalar.copy(fb[:, s0:s1], a8g[:, s0:s1])
                            s0 = s1
                    else:
                        nc.gpsimd.tensor_copy(fb[:, c0:c1], a8g[:, c0:c1])
                    c0 = c1
                return fb

            ng = len(grps)
            for oi, gi in enumerate(range(ng)):
                k0, k1 = grps[gi]
                fb = cast_group_f16(gi)
                if oi < ng - 1:
                    for k in range(k0, k1):
                        kk = k - k0
                        nr = _rows(k)
                        rhs = x0s[0:nr, k * 128:(k + 1) * 128]
                        for t in range(10):
                            ci, tt = (t // 4, t % 4)
                            nc.tensor.matmul(
                                psum1[ci][:, tt * 128:(tt + 1) * 128],
                                fb[0:nr, kk * SP + t * 128:
                                   kk * SP + (t + 1) * 128],
                                rhs,
                                start=False, stop=False,
                            )
                else:
                    for t in range(10):
                        ci, tt = (t // 4, t % 4)
                        for k in range(k0, k1):
                            kk = k - k0
                            nr = _rows(k)
                            last_mm = nc.tensor.matmul(
                                psum1[ci][:, tt * 128:(tt + 1) * 128],
                                fb[0:nr, kk * SP + t * 128:
                                   kk * SP + (t + 1) * 128],
                                x0s[0:nr, k * 128:(k + 1) * 128],
                                start=False,
                                stop=(k == k1 - 1 and t in (3, 7, 9)),
                            )

            # keep-warm matmuls + L2 psum seeds (full-bank tiles)
            psumd = ps.tile([128, 512], f32, tag="warmups")
            psum2t = []
            for ti, cn in enumerate((512, 512, 256)):
                p2t = ps.tile([128, cn], f32, tag=f"acc2_{ti}", name=f"p2_{ti}")
                psum2t.append(p2t)

            def p2slice(ci, cn):
                return psum2t[ci // 2][:, (ci % 2) * 256:(ci % 2) * 256 + cn]

            lastl1 = _NameSet([last_mm.ins.name])
            for ti, cn in enumerate((512, 512, 256)):
                smm = nc.tensor.matmul(
                    psum2t[ti][:, 0:cn], zl[:, 0:128], zl[:, 0:cn],
                    start=True, stop=False,
                )
                smm.ins.add_sync_dependencies_from(lastl1)
            for _ in range(ndummy):
                dmm = nc.tensor.matmul(
                    psumd[:], zl[:, 0:128], zl[:, 0:512],
                    start=True, stop=True, skip_group_check=True,
                )
                dmm.ins.add_sync_dependencies_from(lastl1)

            # ---- boundary: tanh eviction into fp8 h limbs [hi][lo] ----
            tanh_names = []
            hlimb_names = []
            chunk_evnames = []
            with tc.high_priority():
                for ci, (c0, cn) in enumerate(CHUNKS):
                    th = thp.tile([128, 512], f16, tag="th")
                    t_i = nc.scalar.activation(
                        th[:, 0:cn], psum1[ci][:, 0:cn],
                        mybir.ActivationFunctionType.Tanh,
                        scale=cscs[:, ci:ci + 1],
                    )
                    tanh_names.append(t_i.ins.name)
                    h_i = nc.vector.tensor_copy(
                        agin_sb[:, c0:c0 + cn], th[:, 0:cn])
                    l_i = nc.gpsimd.tensor_sub(
                        agin_sb[:, SP + c0:SP + c0 + cn],
                        th[:, 0:cn], agin_sb[:, c0:c0 + cn])
                    hlimb_names += [h_i.ins.name, l_i.ins.name]
                    chunk_evnames.append(
                        [t_i.ins.name, h_i.ins.name, l_i.ins.name])
                nc.sync.dma_start(agin[:], agin_sb[:])

                if nocc:
                    ag_inst = nc.sync.dma_start(agout[0:128, :], agin[:])
                else:
                    ag_inst = nc.gpsimd.collective_compute(
                        "AllGather",
                        mybir.AluOpType.bypass,
                        replica_groups=[list(range(NCORES))],
                        ins=[agin.opt()],
                        outs=[agout.opt()],
                    )
                agdep = _NameSet([ag_inst.ins.name])
                pid = nc.sync.partition_id()
                x1names = [[] for _ in range(NCORES)]

                def emit_span(g0):
                    nkt = 2 * RSPAN
                    soff0 = (g0 % NCR) * 2 * 2 * SP
                    rd = nc.sync.dma_start(
                        crs[:, soff0:soff0 + nkt * 2 * SP].rearrange(
                            "p (k two j) -> p k two j", two=2, j=SP
                        )[:, :, 1, :],
                        ar[2 * g0:2 * g0 + nkt].rearrange("k p j -> p k j"),
                    )
                    pre_rdma[g0] = rd.ins.name

                for i in range(1, NCORES):
                    # keep the r-ring fed just ahead of x1 deliveries
                    if i in (1, 3, 5):
                        emit_span(2 + 2 * i)
                    boff = ((pid + i) % NCORES) * (128 * 2 * SP)
                    base = (i - 1) * 2 * SP
                    dsrc = agout[0:128, :].copy()
                    dsrc.offset = boff
                    xd = nc.sync.dma_start(
                        x1s[:, base:base + 2 * SP], dsrc)
                    xd.ins.add_sync_dependencies_from(agdep)
                    x1names[i].append(xd.ins.name)

            # ---- layer 2 (DoubleRow fp8, exact u8 via (c, r) planes) ----
            def limb_pair(g, nr, which):
                """(h_limb[2g], h_limb[2g+1]) as [p, 2, 128], stride 128.
                which: 0 = h_hi, 1 = h_lo."""
                blk, jj = divmod(g, 5)
                src, base = (agin_sb, 0) if blk == 0 else \
                    (x1s, (blk - 1) * 2 * SP)
                o = base + which * SP + jj * 256
                return src[0:nr, o:o + 256].rearrange(
                    "p (two f) -> p two f", two=2)

            def h_deps(g):
                blk = g // 5
                if blk == 0:
                    # own rank: only the psum chunk covering cols 256g..+256
                    ci = (0, 0, 1, 1, 2)[g]
                    return chunk_evnames[ci]
                return x1names[blk]

            npair = 30              # DR pairs (kt 0-59); kt 60-79 go f16
            crmm = [[] for _ in range(npair)]
            tailinfo = []
            rdma_name = None
            for g in range(npair):
                j = 2 * g
                slot = g % NCR
                soff = slot * 2 * 2 * SP
                prod = []
                derive_r = False
                if g % RSPAN == 0:
                    if g in pre_rdma:
                        rdma_name = pre_rdma[g]
                    else:
                        # one r-DMA covering pair-groups g..g+RSPAN-1
                        gend = min(g + RSPAN, npair)
                        nkt = 2 * (gend - g)
                        war = []
                        for gg in range(g, gend):
                            if gg >= NCR:
                                war += crmm[gg - NCR]
                        rd = nc.sync.dma_start(
                            crs[:, soff:soff + nkt * 2 * SP].rearrange(
                                "p (k two j) -> p k two j", two=2, j=SP
                            )[:, :, 1, :],
                            ar[j:j + nkt].rearrange("k p j -> p k j"),
                        )
                        if war:
                            rd.ins.add_sync_dependencies_from(_NameSet(war))
                        rdma_name = rd.ins.name
                if not derive_r:
                    prod.append(rdma_name)
                war2 = _NameSet(crmm[g - NCR]) if g >= NCR else None
                gi, koff = _pair_src(j)
                a8g = a8_tiles[gi]
                for kk in range(2):
                    for e, s0, s1 in CSPANS:
                        dst = crs[:, soff + kk * 2 * SP + s0:
                                  soff + kk * 2 * SP + s1]
                        srcq = a8g[:, (koff + kk) * SP + s0:
                                   (koff + kk) * SP + s1]
                        if e == "a":
                            if g < 3:
                                # ACT is busy with the tanh chain early on
                                cin = nc.vector.tensor_copy(dst, srcq)
                            else:
                                cin = nc.scalar.copy(dst, srcq)
                                cin.ins.add_sync_dependencies_from(
                                    _NameSet(tanh_names))
                        elif e == "p":
                            cin = nc.gpsimd.tensor_copy(dst, srcq)
                        else:
                            cin = nc.vector.tensor_copy(dst, srcq)
                        if war2 is not None:
                            cin.ins.add_sync_dependencies_from(war2)
                        prod.append(cin.ins.name)
                    if derive_r:
                        # r = q - c on DVE/Pool (ACT cannot STT)
                        rdst = crs[:, soff + kk * 2 * SP + SP:
                                   soff + (kk + 1) * 2 * SP]
                        rsrc0 = a8g[:, (koff + kk) * SP:(koff + kk + 1) * SP]
                        rsrc1 = crs[:, soff + kk * 2 * SP:
                                    soff + kk * 2 * SP + SP]
                        eng = nc.vector if (g + kk) % 2 == 0 else nc.gpsimd
                        rin = eng.scalar_tensor_tensor(
                            rdst, rsrc0, 1.0, rsrc1,
                            op0=mybir.AluOpType.mult,
                            op1=mybir.AluOpType.subtract)
                        if war2 is not None:
                            rin.ins.add_sync_dependencies_from(war2)
                        prod.append(rin.ins.name)
                pdep = _NameSet(prod)
                hdep = _NameSet(h_deps(g))
                nr = min(_rows(j), _rows(j + 1))
                crg = crs[0:nr, soff:soff + 2 * 2 * SP].rearrange(
                    "p (kk two c) -> p kk two c", two=2, c=SP)
                if g < npair - 5:
                    for ci, (c0, cn) in enumerate(CH2):
                        for limb, plane in ((0, 0), (0, 1), (1, 0)):
                            mm = nc.tensor.matmul(
                                p2slice(ci, cn),
                                limb_pair(g, nr, limb),
                                crg[:, :, plane, c0:c0 + cn],
                                start=False, stop=False,
                                perf_mode=DR,
                            )
                            mm.ins.add_sync_dependencies_from(pdep)
                            mm.ins.add_sync_dependencies_from(hdep)
                            crmm[g].append(mm.ins.name)
                else:
                    tailinfo.append((g, nr, crg, pdep, hdep))
            # final rank block chunk-outer: each psum tile stops (and its
            # eviction starts) as soon as its own chunks' matmuls finish
            for ci in (4, 0, 1, 2, 3):
                c0, cn = CH2[ci]
                for g, nr, crg, pdep, hdep in tailinfo:
                    for mi, (limb, plane) in enumerate(
                            ((0, 0), (0, 1), (1, 0))):
                        mm = nc.tensor.matmul(
                            p2slice(ci, cn),
                            limb_pair(g, nr, limb),
                            crg[0:nr, :, plane, c0:c0 + cn],
                            start=False, stop=False,
                            perf_mode=DR,
                        )
                        mm.ins.add_sync_dependencies_from(pdep)
                        mm.ins.add_sync_dependencies_from(hdep)
                        crmm[g].append(mm.ins.name)

            # f16 tail of layer 2: k-tiles 60..79 (L1 cast groups 16..20)
            # exact h = h_hi + h_lo summed once into f16
            h16adds = []
            for bi, blk in enumerate((6, 7)):
                base = (blk - 1) * 2 * SP
                ad = nc.vector.tensor_add(
                    h16[:, bi * SP:(bi + 1) * SP],
                    x1s[:, base:base + SP],
                    x1s[:, base + SP:base + 2 * SP])
                ad.ins.add_sync_dependencies_from(_NameSet(x1names[blk]))
                h16adds.append(ad.ins.name)
            h16dep = _NameSet(h16adds)
            f16_g0 = 16
            for gi in range(f16_g0, len(grps)):
                k0, k1 = grps[gi]
                fb2 = cast_group_f16(gi)
                for ci, (c0, cn) in enumerate(CHUNKS):
                    for k in range(k0, k1):
                        kk = k - k0
                        nr = _rows(k)
                        mm = nc.tensor.matmul(
                            psum2t[ci][:, 0:cn],
                            h16[0:nr, (k - 60) * 128:(k - 59) * 128],
                            fb2[0:nr, kk * SP + c0:kk * SP + c0 + cn],
                            start=False,
                            stop=(gi == len(grps) - 1 and k == k1 - 1),
                        )
                        mm.ins.add_sync_dependencies_from(h16dep)

            # evict layer 2 per psum tile: ob = psum2 [* W1] -> out
            dq_eng = (nc.sync, nc.sync, nc.scalar)
            with tc.high_priority():
                for ti, (c0, cn) in enumerate(CHUNKS):
                    if w1_ones:
                        if ti == 1:
                            nc.scalar.copy(
                                ob[:, c0:c0 + cn], psum2t[ti][:, 0:cn])
                        else:
                            nc.vector.tensor_copy(
                                ob[:, c0:c0 + cn], psum2t[ti][:, 0:cn])
                    else:
                        nc.vector.tensor_scalar(
                            ob[:, c0:c0 + cn], psum2t[ti][:, 0:cn],
                            w1s[:, 0:1], None, mybir.AluOpType.mult)
                    dq_eng[ti].dma_start(
                        out[:, c0:c0 + cn], ob[:, c0:c0 + cn]
                    )

    nc.compile()
    return nc


def get_program(nocc=False, gsizes=GSIZES, ncast=NCAST, w1_ones=True,
                ndummy=NDUMMY):
    key = ("nc", nocc, tuple(gsizes), ncast, w1_ones, ndummy)
    if key not in _PROG_CACHE:
        _PROG_CACHE[key] = _build_program(nocc, gsizes, ncast, w1_ones,
                                          ndummy)
    return _PROG_CACHE[key]


def _slot_order():
    slots = np.empty(SP, np.int64)
    i = 0
    for ci, (tile0, ntile) in enumerate(((0, 4), (4, 4), (8, 2))):
        for p in range(128):
            for ti in range(ntile):
                slots[i] = (tile0 + ti) * 128 + p
                i += 1
    assert i == SP
    return slots


_SLOTS = _slot_order()


def _core_perm(colmax_ext):
    order = np.argsort(-colmax_ext, kind="stable")
    perm = np.empty(SP, np.int64)
    perm[_SLOTS] = order
    return perm


def build_in_maps(x, src, dst, vals, W):
    """Host-side prep: u8-quantized A^T shard (4 sorted columns per scale
    group, ceiling 247) + fp8 r-residual stream + f16 x0, in the per-core
    permuted slot order."""
    import scipy.sparse as sp
    import ml_dtypes

    F8 = ml_dtypes.float8_e4m3

    x = np.asarray(x, np.float32)
    src = np.asarray(src, np.int64)
    dst = np.asarray(dst, np.int64)
    vals = np.asarray(vals, np.float32)
    W = np.asarray(W, np.float32)

    AT = sp.coo_matrix((vals, (src, dst)), shape=(N, N)).toarray()

    perms = []
    steps = []
    cscs = []
    for c in range(NCORES):
        ATc = AT[:, c * S:(c + 1) * S]
        colmax_ext = np.full(SP, -1.0, np.float32)
        colmax_ext[:S] = ATc.max(axis=0)
        perm = _core_perm(colmax_ext)
        cm_slot = np.maximum(colmax_ext[perm], 1e-9)
        step_slot = np.empty(SP, np.float32)
        csc = np.empty((128, 3), np.float32)
        for ci, (tile0, ntile) in enumerate(((0, 4), (4, 4), (8, 2))):
            t_sl = slice(tile0 * 128, (tile0 + ntile) * 128)
            cm = cm_slot[t_sl].reshape(ntile, 128)
            gmax = cm.max(axis=0) / float(QMAX)
            csc[:, ci] = gmax
            step_slot[t_sl] = np.tile(gmax[None, :], (ntile, 1)).reshape(-1)
        perms.append(perm)
        steps.append(step_slot)
        cscs.append(np.ascontiguousarray(csc))

    node2s, valid2s = [], []
    for c in range(NCORES):
        node2 = np.empty(NPAD, np.int64)
        valid2 = np.empty(NPAD, bool)
        for i in range(NCORES):
            r = (c + i) % NCORES
            pr = perms[r]
            valid = pr < S
            node2[i * SP:(i + 1) * SP] = np.where(valid, r * S + pr, 0)
            valid2[i * SP:(i + 1) * SP] = valid
        node2s.append(node2)
        valid2s.append(valid2)

    xw = x * W[0][None, :]
    w1col = np.ascontiguousarray(W[1][:, None]).astype(np.float32)

    in_maps = []
    for c in range(NCORES):
        node2, valid2 = node2s[c], valid2s[c]
        x0p = np.zeros((NPAD, D), np.float32)
        x0p[valid2] = xw[node2[valid2]]
        x0h = np.ascontiguousarray(
            x0p.reshape(KT, 128, D).transpose(1, 0, 2).reshape(128, KT * D)
        ).astype(np.float16)
        ATc = AT[:, c * S:(c + 1) * S]
        perm = perms[c]
        valid = perm < S
        ATs = np.zeros((N, SP), np.float32)
        ATs[:, valid] = ATc[:, perm[valid]]
        Aq = np.clip(np.rint(ATs / steps[c][None, :]), 0, QMAX).astype(
            np.uint8
        )
        Ap = np.zeros((NPAD, SP), Aq.dtype)
        Ap[valid2] = Aq[node2[valid2]]
        a3 = np.ascontiguousarray(Ap.reshape(KT, 128, SP))
        # r = q - RTNE_e4m3(q): integers in [-8, 8], exact in e4m3
        cq = a3.astype(F8).astype(np.float32)
        r3 = np.ascontiguousarray(
            (a3.astype(np.float32) - cq).astype(F8))
        in_maps.append(
            {
                "a": a3,
                "ar": r3,
                "x0": x0h,
                "csc": cscs[c],
                "w1c": w1col,
            }
        )
    return in_maps, (steps, perms)


def assemble_output(results, aux):
    steps, perms = aux
    outs = []
    for c in range(NCORES):
        ot = np.asarray(results[c]["out"], np.float32)
        ot = ot * steps[c][None, :]
        perm = perms[c]
        valid = perm < S
        o = np.zeros((S, 128), np.float32)
        o[perm[valid]] = ot[:, valid].T
        outs.append(o)
    return np.ascontiguousarray(np.concatenate(outs, axis=0))


def kernel(x, src, dst, vals, W):
    from concourse import bass_utils

    w1_ones = bool(np.all(np.asarray(W)[1] == 1.0))
    nc = get_program(w1_ones=w1_ones)
    in_maps, steps = build_in_maps(x, src, dst, vals, W)
    import time as _time

    last_err = None
    for sleep_s in (10.0, 30.0, 60.0, 0.0):
        try:
            res = bass_utils.run_bass_kernel_spmd(
                nc, in_maps, core_ids=list(range(NCORES))
            )
            return assemble_output(res.results, steps)
        except Exception as e:  # noqa: BLE001
            last_err = e
            _time.sleep(sleep_s)
    raise last_err


# revision 28
# speedup vs baseline: 1.0485x; 1.0485x over previous
"""GCN diag-encoder (2-layer SpMM) on 8 Trainium2 NeuronCores.

Strategy: the sparse adjacency (640K edges over 10K nodes, ~0.64% dense) is
materialized as a dense A^T matrix on the host; each per-layer
  out[dst] = sum_e vals[e] * x[src[e]]        (segment-sum SpMM)
becomes dense TensorEngine matmuls.  Each core owns a 1250-wide dst slice of
A^T (padded to 1280, uint8-quantized per dst column).

v3: A^T is DMA'd ONCE as raw uint8 (half the DMA bytes of a u8->f16
cast-DMA, which is charged at the f16 destination size) and stays resident
in SBUF (100KB/partition).  The u8->f16 conversion runs on-chip, split
across the three otherwise-idle compute engines (DVE / Activation / GpSimd)
into a rotating ring of f16 staging tiles that feed the PE.  Both layers
re-cast from the same resident u8 copy, so layer 2 needs no A traffic at
all.  This turns layer 1 from DMA-bound (~93us) into PE-bound (~45us) and
removes layer 2's 29us f16 re-stream.

Layer 1 runs A-stationary — matmul(out=psum[dst,feat], lhsT=AT_tile[src,dst],
rhs=x_tile[src,feat]) — so the layer-1 output is node-major: the eviction is
a fused tanh+dequant-scale pass on the scalar engine straight into the
AllGather bounce.  The host sorts each core's dst columns by quantization
range and packs 4 similar columns per (psum bank, partition) slot, so the
dequant scale is per-partition within a bank and the whole eviction is 3
bank-wide activations (full per-column accuracy at bank-chunk cost).  Each
psum bank is its own tile (psum reads are dependency-tracked whole-tile, so
per-bank tiles let each bank's eviction start at its own stop) and is
seeded by one full-width start=True zero matmul.  Layer 2 runs X-stationary
— matmul(out=psum[feat,dst], lhsT=x1_tile[src,feat], rhs=AT_tile[src,dst]);
its dequant scale and the final un-permute are applied on the host.

Src row-blocks are rotated per core so block 0 is the core's OWN rank:
layer 2's first 10 k-tiles read the tanh output agin_sb directly from SBUF
(no AllGather round-trip), hiding most of the collective latency behind
real work; the other 7 blocks are fetched from the AllGather output at
register-computed offsets ((partition_id+i)%8).  A few zl-by-zl keep-warm
matmuls stop the PE from dropping out of its max p-state across the
remaining gap.  W0 is folded into x on the host; W1 is skipped on device
when it is all-ones (torch init), else applied via a broadcast multiply.
"""

import numpy as np

N = 10000          # nodes
D = 128            # feature dim
NCORES = 8
S = 1250           # dst nodes per core
SP = 1280          # padded dst per core (10 tiles of 128)
KT = 80            # contraction k-tiles (padded src rows = 10240)
NPAD = KT * 128    # 10240
GSIZES = (2, 2) + (4,) * 19      # k-tiles per group (sum = 80)
NCAST = 7          # f16 staging ring depth
NDUMMY = 6         # PE keep-warm matmuls bridging the AllGather valley
# psum bank chunks: layer-1 eviction + layer-2 column blocking
CHUNKS = ((0, 512), (512, 512), (1024, 256))
# the 30 pad columns per core sort to partitions 113..127 of tiles 8 and 9,
# so k-tiles with k%10 in (8,9) only have 113 real src rows
NPADROW = 113


def _rows(k):
    return NPADROW if k % 10 in (8, 9) else 128

_PROG_CACHE = {}


def _build_program(nocc=False, gsizes=GSIZES, ncast=NCAST, w1_ones=True,
                   ndummy=NDUMMY):
    import concourse.bacc as bacc
    import concourse.mybir as mybir
    from bass_rust import InstructionNameOrderedSet as _NameSet
    from concourse import tile

    f32 = mybir.dt.float32
    f16 = mybir.dt.float16
    u8 = mybir.dt.uint8
    grps = []
    _k0 = 0
    for _sz in gsizes:
        grps.append((_k0, _k0 + _sz))
        _k0 += _sz
    assert _k0 == KT
    maxg = max(k1 - k0 for k0, k1 in grps)

    nc = bacc.Bacc(
        "TRN2",
        target_bir_lowering=False,
        debug=False,
        enable_asserts=False,
        num_devices=1 if nocc else NCORES,
    )

    a = nc.dram_tensor("a", [KT, 128, SP], u8, kind="ExternalInput").ap()
    x0 = nc.dram_tensor("x0", [128, NPAD], f16, kind="ExternalInput").ap()
    # per-(bank chunk, partition) dequant scales; the host sorts dst
    # columns by quant range so each (chunk, partition) slot's 4 columns
    # share one scale -> the tanh eviction is 3 bank-wide activations
    csc = nc.dram_tensor("csc", [128, 3], f32, kind="ExternalInput").ap()
    # broadcast W1 row tiled x4 (only read when not w1_ones)
    w1b = nc.dram_tensor("w1b", [128, 512], f16, kind="ExternalInput").ap()
    out = nc.dram_tensor("out", [128, SP], f32, kind="ExternalOutput").ap()

    with tile.TileContext(nc) as tc:
        with (
            tc.tile_pool(name="xp", bufs=1) as xp,
            tc.tile_pool(name="a8p", bufs=1) as a8p,
            tc.tile_pool(name="fc", bufs=ncast) as fcp,
            tc.tile_pool(name="ps", bufs=1, space="PSUM") as ps,
            tc.tile_pool(name="dr", bufs=1, space="DRAM") as dr,
        ):
            # x0 is dead once layer 1 finishes; share one slot for both
            x0s = xp.tile([128, NPAD], f16, tag="xs")
            x1s = xp.tile([128, NPAD], f16, tag="xs")
            cscs = xp.tile([128, 3], f32, tag="cscs")
            w1s = xp.tile([128, 512], f16, tag="w1s")
            zl = xp.tile([128, 512], f16, tag="zl")
            warm = xp.tile([128, 1], f32, tag="warm")
            nc.scalar.dma_start(cscs[:], csc)
            if not w1_ones:
                nc.scalar.dma_start(w1s[:], w1b)
            nc.vector.memset(zl[:], 0.0)
            # pre-load the ACT tanh table so the layer-1 eviction doesn't
            # pay the table load on the critical path
            nc.scalar.activation(
                warm[:], zl[:, 0:1], mybir.ActivationFunctionType.Tanh
            )

            agin = dr.tile([128, SP], f16)
            agout = dr.tile([NCORES * 128, SP], f16, addr_space="Shared")

            a8_tiles = {}

            def cast_group(gi, engines="vap", deps=None):
                """u8 -> f16 of resident group gi, split over the engines in
                `engines` (v=DVE, a=ACT in ~1us slices, p=POOL), shares
                proportional to their elementwise rates.  `deps` maps an
                engine letter to instruction names the slice must follow —
                used at the layer boundary so the scheduler cannot hoist
                casts ahead of the tanh -> AllGather chain."""
                k0, k1 = grps[gi]
                w = (k1 - k0) * SP
                a8 = a8_tiles[gi]
                fb = fcp.tile([128, maxg * SP], f16, tag="fc")
                rates = {"v": 4, "a": 4, "p": 3}
                tot = sum(rates[e] for e in engines)
                halves = deps.pop("halves", 1) if deps else 1
                bounds = [(w * h) // halves for h in range(halves + 1)]
                for h0, h1 in zip(bounds, bounds[1:]):
                  wh = h1 - h0
                  c0 = h0
                  for e in engines:
                    c1 = h1 if e == engines[-1] else c0 + (wh * rates[e]) // tot
                    insts = []
                    if e == "v":
                        insts.append(
                            nc.vector.tensor_copy(fb[:, c0:c1], a8[:, c0:c1])
                        )
                    elif e == "a":
                        # <=1.3k-elem slices so ACT never blocks the layer-1
                        # eviction chain behind a long copy
                        s0 = c0
                        while s0 < c1:
                            s1 = min(s0 + 1280, c1)
                            insts.append(
                                nc.scalar.copy(fb[:, s0:s1], a8[:, s0:s1])
                            )
                            s0 = s1
                    else:
                        insts.append(
                            nc.gpsimd.tensor_copy(fb[:, c0:c1], a8[:, c0:c1])
                        )
                    if deps and e in deps:
                        for inst in insts:
                            inst.ins.add_sync_dependencies_from(_NameSet(deps[e]))
                    c0 = c1
                return fb

            # ---- layer 1 (A-stationary; psum is [dst slot, feat]) ----
            # one psum tile per 2KiB bank: psum reads are dependency-tracked
            # whole-tile, so per-bank tiles let each bank's eviction start at
            # its own stop instead of after the layer's last matmul
            psum1 = []
            for ci, (c0, cn) in enumerate(CHUNKS):
                p1t = ps.tile([128, cn], f32, tag=f"acc1_{ci}", name=f"p1_{ci}")
                psum1.append(p1t)
            for ci, (c0, cn) in enumerate(CHUNKS):
                nc.tensor.matmul(
                    psum1[ci][:, 0:cn], zl[:, 0:128], zl[:, 0:cn],
                    start=True, stop=False,
                )
            # x0 for the first four groups rides ahead of their a8 loads so
            # the DMA queue can stay a couple of groups in front of the PE
            xlead = grps[3][1] * 128
            for gi, (k0, k1) in enumerate(grps):
                a8 = a8p.tile([128, (k1 - k0) * SP], u8, tag=f"a8_{gi}")
                a8_tiles[gi] = a8
                kb = {k0, k1}
                if 3 <= gi <= 6 and (k1 - k0) % 2 == 0:
                    kb.add((k0 + k1) // 2)
                for k in range(k0, k1 + 1):
                    if k % 10 in (8, 9) and k0 < k < k1 and _rows(k - 1) != \
                            NPADROW:
                        kb.add(k)
                    if k % 10 == 0 and k0 < k < k1 and _rows(k - 1) == \
                            NPADROW:
                        kb.add(k)
                kb = sorted(kb)
                for b0, b1 in zip(kb, kb[1:]):
                    nr = NPADROW if _rows(b0) == NPADROW else 128
                    nc.sync.dma_start(
                        a8[0:nr, (b0 - k0) * SP:(b1 - k0) * SP].rearrange(
                            "p (k j) -> p k j", k=b1 - b0
                        ),
                        a[b0:b1, 0:nr].rearrange("k p j -> p k j"),
                    )
                if gi == 0:
                    nc.sync.dma_start(x0s[:, 0:xlead], x0[:, 0:xlead])
                if gi >= 4:
                    nc.sync.dma_start(
                        x0s[:, k0 * 128:k1 * 128], x0[:, k0 * 128:k1 * 128]
                    )
            ng = len(grps)
            for oi, gi in enumerate(range(ng)):
                k0, k1 = grps[gi]
                fb = cast_group(gi, deps={"halves": 2}
                                if 3 <= gi <= 6 else None)
                if oi < ng - 1:
                    for k in range(k0, k1):
                        kk = k - k0
                        nr = _rows(k)
                        rhs = x0s[0:nr, k * 128:(k + 1) * 128]
                        for t in range(10):
                            ci, tt = (t // 4, t % 4)
                            nc.tensor.matmul(
                                psum1[ci][:, tt * 128:(tt + 1) * 128],
                                fb[0:nr, kk * SP + t * 128:
                                   kk * SP + (t + 1) * 128],
                                rhs,
                                start=False, stop=False,
                            )
                else:
                    # final group t-outer: each dst range finishes early so
                    # the tanh eviction overlaps the remaining matmuls
                    for t in range(10):
                        ci, tt = (t // 4, t % 4)
                        for k in range(k0, k1):
                            kk = k - k0
                            nr = _rows(k)
                            last_mm = nc.tensor.matmul(
                                psum1[ci][:, tt * 128:(tt + 1) * 128],
                                fb[0:nr, kk * SP + t * 128:
                                   kk * SP + (t + 1) * 128],
                                x0s[0:nr, k * 128:(k + 1) * 128],
                                start=False,
                                stop=(k == k1 - 1 and t in (3, 7, 9)),
                            )

            # evict layer 1: x1 = tanh(cs_dst * psum1) [* W1] on ACT; DMA to
            # the AllGather bounce per psum bank so agin lands early.  The
            # whole tanh -> agin -> AllGather -> x1s chain is the only work
            # between the two PE-bound layers, so it runs at high priority
            # and its DMAs ride the otherwise-idle SP queue.
            agin_sb = xp.tile([128, SP], f16, tag="agin")
            # keep-warm matmuls: PE would otherwise idle across the AllGather
            # valley and restart cold (2.4x slower for the first 3us)
            psumd = ps.tile([128, 512], f32, tag="warmups")
            for _ in range(ndummy):
                dmm = nc.tensor.matmul(
                    psumd[:], zl[:, 0:128], zl[:, 0:512],
                    start=True, stop=True, skip_group_check=True,
                )
                # pin behind layer 1 so the scheduler cannot hoist the
                # warm-up matmuls to the (DMA-bound) start of the program
                dmm.ins.add_sync_dependencies_from(_NameSet([last_mm.ins.name]))
            tanh_last = None
            with tc.high_priority():
                for ci, (c0, cn) in enumerate(CHUNKS):
                    tanh_last = nc.scalar.activation(
                        agin_sb[:, c0:c0 + cn], psum1[ci][:, 0:cn],
                        mybir.ActivationFunctionType.Tanh,
                        scale=cscs[:, ci:ci + 1],
                    )
                    if not w1_ones:
                        nc.vector.tensor_mul(
                            agin_sb[:, c0:c0 + cn], agin_sb[:, c0:c0 + cn],
                            w1s[:, 0:cn]
                        )
                    nc.sync.dma_start(
                        agin[:, c0:c0 + cn], agin_sb[:, c0:c0 + cn]
                    )

                if nocc:
                    ag_inst = nc.sync.dma_start(agout[0:128, :], agin[:])
                else:
                    ag_inst = nc.gpsimd.collective_compute(
                        "AllGather",
                        mybir.AluOpType.bypass,
                        replica_groups=[list(range(NCORES))],
                        ins=[agin.opt()],
                        outs=[agout.opt()],
                    )
                # A's src row-blocks are rotated per core so block 0 is the
                # core's OWN rank: layer 2's first 10 k-tiles read agin_sb
                # directly (no AllGather round-trip), and block i (i>=1) is
                # rank (pid+i)%8, fetched from agout at a register-computed
                # offset.
                agdep = _NameSet([ag_inst.ins.name])
                pid = nc.sync.partition_id()
                x1dmas = []
                for i in range(1, NCORES):
                    boff = ((pid + i) % NCORES) * (128 * SP)
                    if i == 1:
                        # lead slice so k-tile 10 can start while the rest
                        # of the block is in flight
                        dsrc = agout[0:128, 0:384].copy()
                        dsrc.offset = boff
                        x1dmas.append(
                            nc.sync.dma_start(x1s[:, SP:SP + 384], dsrc)
                        )
                        dsrc = agout[0:128, 384:SP].copy()
                        dsrc.offset = boff + 384
                        x1dmas.append(
                            nc.sync.dma_start(
                                x1s[:, SP + 384:2 * SP], dsrc
                            )
                        )
                    else:
                        dsrc = agout[0:128, :].copy()
                        dsrc.offset = boff
                        x1dmas.append(
                            nc.sync.dma_start(
                                x1s[:, i * SP:(i + 1) * SP], dsrc
                            )
                        )
                # DRAM->SBUF reads of the collective output are not tracked
                # as data deps in the single-core twin; pin them so the
                # scheduler cannot float them ahead of the agin writes
                for d in x1dmas:
                    d.ins.add_sync_dependencies_from(agdep)

            # ---- layer 2 (X-stationary; psum is [feat, dst]) ----
            # All of A is already resident as u8; only the casts re-run.
            # The first ring of casts has no x1 dependency, so it completes
            # during the AllGather and PE starts as soon as rank 0 lands.
            psum2 = []
            for ci, (c0, cn) in enumerate(CHUNKS):
                p2t = ps.tile([128, cn], f32, tag=f"acc2_{ci}", name=f"p2_{ci}")
                psum2.append(p2t)
            ob = xp.tile([128, SP], f32, tag="ob")

            def lhsT_of(k):
                # row-block 0 is the core's own rank: its activations are
                # already on-chip in agin_sb (same [dst slot, feat] layout)
                nr = _rows(k)
                if k < 10:
                    return agin_sb[0:nr, k * 128:(k + 1) * 128]
                return x1s[0:nr, k * 128:(k + 1) * 128]

            first = True
            tdep = [tanh_last.ins.name]
            adep = [ag_inst.ins.name]
            for gi, (k0, k1) in enumerate(grps):
                # keep ACT free for the tanh chain and POOL free for the
                # AllGather issue while the boundary groups pre-cast on DVE;
                # ACT/POOL rejoin once their part of the chain retires
                if gi < 3:
                    fb = cast_group(gi, "v")
                elif gi < 5:
                    fb = cast_group(gi, "va", deps={"a": tdep})
                elif gi < 9:
                    fb = cast_group(gi, "vap", deps={"a": tdep, "p": adep})
                else:
                    fb = cast_group(gi)
                last_grp = gi == len(grps) - 1
                if not last_grp:
                    for k in range(k0, k1):
                        kk = k - k0
                        lhsT = lhsT_of(k)
                        nr = _rows(k)
                        for ci, (c0, cn) in enumerate(CHUNKS):
                            nc.tensor.matmul(
                                psum2[ci][:, 0:cn],
                                lhsT,
                                fb[0:nr, kk * SP + c0: kk * SP + c0 + cn],
                                start=first, stop=False,
                            )
                        first = False
                else:
                    # final group: bank-outer with per-bank stops; ALL
                    # evictions are created after the matmuls (psum reads
                    # are tracked whole-tile, so an earlier-created read
                    # would falsely serialize the later banks' matmuls)
                    for ci, (c0, cn) in enumerate(CHUNKS):
                        for k in range(k0, k1):
                            kk = k - k0
                            nc.tensor.matmul(
                                psum2[ci][:, 0:cn],
                                lhsT_of(k),
                                fb[0:_rows(k), kk * SP + c0:
                                   kk * SP + c0 + cn],
                                start=False, stop=(k == k1 - 1),
                            )
                    # GPSIMD cannot read PSUM on HW: evict banks on DVE/ACT
                    dq_eng = (nc.sync, nc.sync, nc.scalar)
                    with tc.high_priority():
                        for ci, (c0, cn) in enumerate(CHUNKS):
                            if ci == 1:
                                nc.scalar.copy(
                                    ob[:, c0:c0 + cn], psum2[ci][:, 0:cn]
                                )
                            else:
                                nc.vector.tensor_copy(
                                    ob[:, c0:c0 + cn], psum2[ci][:, 0:cn]
                                )
                            dq_eng[ci].dma_start(
                                out[:, c0:c0 + cn], ob[:, c0:c0 + cn]
                            )

    nc.compile()
    return nc


def get_program(nocc=False, gsizes=GSIZES, ncast=NCAST, w1_ones=True,
                ndummy=NDUMMY):
    key = ("nc", nocc, tuple(gsizes), ncast, w1_ones, ndummy)
    if key not in _PROG_CACHE:
        _PROG_CACHE[key] = _build_program(nocc, gsizes, ncast, w1_ones,
                                          ndummy)
    return _PROG_CACHE[key]


def _slot_order():
    """Slot s = t*128 + p (tile t in 0..9, partition p) listed in quant-sort
    order: chunks of 4 (banks 0/1) or 2 (bank 2) consecutive sorted columns
    share one (chunk, partition) slot group, hence one dequant scale."""
    slots = np.empty(SP, np.int64)
    i = 0
    for ci, (tile0, ntile) in enumerate(((0, 4), (4, 4), (8, 2))):
        for p in range(128):
            for ti in range(ntile):
                slots[i] = (tile0 + ti) * 128 + p
                i += 1
    assert i == SP
    return slots


_SLOTS = _slot_order()


def _core_perm(colmax_ext):
    """perm[s] = original local dst column (or >=S for pad) in slot s, with
    columns sorted by quant range so slot groups share a scale."""
    order = np.argsort(-colmax_ext, kind="stable")  # [SP] sorted col ids
    perm = np.empty(SP, np.int64)
    perm[_SLOTS] = order
    return perm


def build_in_maps(x, src, dst, vals, W):
    """Host-side prep: dense A^T shard (u8 quantized, 4 sorted columns per
    scale group) + x0, both in the per-core permuted slot order."""
    import scipy.sparse as sp

    x = np.asarray(x, np.float32)
    src = np.asarray(src, np.int64)
    dst = np.asarray(dst, np.int64)
    vals = np.asarray(vals, np.float32)
    W = np.asarray(W, np.float32)

    # A[dst, src] = sum of vals  ->  we build AT[src, dst]
    AT = sp.coo_matrix((vals, (src, dst)), shape=(N, N)).toarray()

    # per-core column permutations (dst side of A, src rows of A, x rows)
    perms = []
    steps = []
    cscs = []
    for c in range(NCORES):
        ATc = AT[:, c * S:(c + 1) * S]  # [N, S] float32
        colmax_ext = np.full(SP, -1.0, np.float32)
        colmax_ext[:S] = ATc.max(axis=0)
        perm = _core_perm(colmax_ext)
        # group scale = max colmax over each slot group (same (chunk, p))
        cm_slot = np.maximum(colmax_ext[perm], 1e-9)  # [SP] by slot
        step_slot = np.empty(SP, np.float32)
        csc = np.empty((128, 3), np.float32)
        for ci, (tile0, ntile) in enumerate(((0, 4), (4, 4), (8, 2))):
            t_sl = slice(tile0 * 128, (tile0 + ntile) * 128)
            cm = cm_slot[t_sl].reshape(ntile, 128)    # [ntile, p]
            gmax = cm.max(axis=0) / 255.0             # [p]
            csc[:, ci] = gmax
            step_slot[t_sl] = np.tile(gmax[None, :], (ntile, 1)).reshape(-1)
        perms.append(perm)
        steps.append(step_slot)
        cscs.append(np.ascontiguousarray(csc))

    # per-core src slot -> node mapping: row-block i of core c is rank
    # (c+i)%8 (own rank first, so layer 2 starts from on-chip activations),
    # permuted within the block by that rank's own column permutation
    node2s, valid2s = [], []
    for c in range(NCORES):
        node2 = np.empty(NPAD, np.int64)
        valid2 = np.empty(NPAD, bool)
        for i in range(NCORES):
            r = (c + i) % NCORES
            pr = perms[r]
            valid = pr < S
            node2[i * SP:(i + 1) * SP] = np.where(valid, r * S + pr, 0)
            valid2[i * SP:(i + 1) * SP] = valid
        node2s.append(node2)
        valid2s.append(valid2)

    xw = x * W[0][None, :]

    w1brow = np.ascontiguousarray(
        np.tile(W[1][None, :], (128, 4))
    ).astype(np.float16)

    in_maps = []
    for c in range(NCORES):
        node2, valid2 = node2s[c], valid2s[c]
        x0p = np.zeros((NPAD, D), np.float32)
        x0p[valid2] = xw[node2[valid2]]
        x0h = np.ascontiguousarray(
            x0p.reshape(KT, 128, D).transpose(1, 0, 2).reshape(128, KT * D)
        ).astype(np.float16)
        ATc = AT[:, c * S:(c + 1) * S]  # [N, S] float32
        perm = perms[c]
        valid = perm < S
        ATs = np.zeros((N, SP), np.float32)
        ATs[:, valid] = ATc[:, perm[valid]]           # columns in slot order
        Aq = np.clip(np.rint(ATs / steps[c][None, :]), 0, 255).astype(
            np.uint8
        )
        Ap = np.zeros((NPAD, SP), Aq.dtype)
        Ap[valid2] = Aq[node2[valid2]]                # rows in slot order
        a3 = np.ascontiguousarray(Ap.reshape(KT, 128, SP))
        in_maps.append(
            {
                "a": a3,
                "x0": x0h,
                "csc": cscs[c],
                "w1b": w1brow,
            }
        )
    return in_maps, (steps, perms)


def assemble_output(results, aux):
    steps, perms = aux
    outs = []
    for c in range(NCORES):
        ot = np.asarray(results[c]["out"], np.float32)  # [128, SP] feat-major
        ot = ot * steps[c][None, :]  # per-dst dequant (layer-2)
        perm = perms[c]
        valid = perm < S
        o = np.zeros((S, 128), np.float32)
        o[perm[valid]] = ot[:, valid].T             # un-permute dst slots
        outs.append(o)
    return np.ascontiguousarray(np.concatenate(outs, axis=0))


def kernel(x, src, dst, vals, W):
    from concourse import bass_utils

    w1_ones = bool(np.all(np.asarray(W)[1] == 1.0))
    nc = get_program(w1_ones=w1_ones)
    in_maps, steps = build_in_maps(x, src, dst, vals, W)
    # The axon terminal can wedge when a different program was loaded
    # earlier in its lifetime; after the crash the terminal restarts and a
    # retry succeeds.  Back off progressively to ride out the restart.
    import time as _time

    last_err = None
    for sleep_s in (10.0, 30.0, 60.0, 0.0):
        try:
            res = bass_utils.run_bass_kernel_spmd(
                nc, in_maps, core_ids=list(range(NCORES))
            )
            return assemble_output(res.results, steps)
        except Exception as e:  # noqa: BLE001
            last_err = e
            _time.sleep(sleep_s)
    raise last_err


# revision 33
# speedup vs baseline: 1.0517x; 1.0031x over previous
"""GCN diag-encoder (2-layer SpMM) on 8 Trainium2 NeuronCores.

Strategy: the sparse adjacency (640K edges over 10K nodes, ~0.64% dense) is
materialized as a dense A^T matrix on the host; each per-layer
  out[dst] = sum_e vals[e] * x[src[e]]        (segment-sum SpMM)
becomes dense TensorEngine matmuls.  Each core owns a 1250-wide dst slice of
A^T (padded to 1280, uint8-quantized per dst column).

v3: A^T is DMA'd ONCE as raw uint8 (half the DMA bytes of a u8->f16
cast-DMA, which is charged at the f16 destination size) and stays resident
in SBUF (100KB/partition).  The u8->f16 conversion runs on-chip, split
across the three otherwise-idle compute engines (DVE / Activation / GpSimd)
into a rotating ring of f16 staging tiles that feed the PE.  Both layers
re-cast from the same resident u8 copy, so layer 2 needs no A traffic at
all.  This turns layer 1 from DMA-bound (~93us) into PE-bound (~45us) and
removes layer 2's 29us f16 re-stream.

Layer 1 runs A-stationary — matmul(out=psum[dst,feat], lhsT=AT_tile[src,dst],
rhs=x_tile[src,feat]) — so the layer-1 output is node-major: the eviction is
a fused tanh+dequant-scale pass on the scalar engine straight into the
AllGather bounce.  The host sorts each core's dst columns by quantization
range and packs 4 similar columns per (psum bank, partition) slot, so the
dequant scale is per-partition within a bank and the whole eviction is 3
bank-wide activations (full per-column accuracy at bank-chunk cost).  Each
psum bank is its own tile (psum reads are dependency-tracked whole-tile, so
per-bank tiles let each bank's eviction start at its own stop) and is
seeded by one full-width start=True zero matmul.  Layer 2 runs X-stationary
— matmul(out=psum[feat,dst], lhsT=x1_tile[src,feat], rhs=AT_tile[src,dst]);
its dequant scale and the final un-permute are applied on the host.

Src row-blocks are rotated per core so block 0 is the core's OWN rank:
layer 2's first 10 k-tiles read the tanh output agin_sb directly from SBUF
(no AllGather round-trip), hiding most of the collective latency behind
real work; the other 7 blocks are fetched from the AllGather output at
register-computed offsets ((partition_id+i)%8).  A few zl-by-zl keep-warm
matmuls stop the PE from dropping out of its max p-state across the
remaining gap.  W0 is folded into x on the host; W1 is skipped on device
when it is all-ones (torch init), else applied via a broadcast multiply.
"""

import numpy as np

N = 10000          # nodes
D = 128            # feature dim
NCORES = 8
S = 1250           # dst nodes per core
SP = 1280          # padded dst per core (10 tiles of 128)
KT = 80            # contraction k-tiles (padded src rows = 10240)
NPAD = KT * 128    # 10240
GSIZES = (2, 2) + (4,) * 19      # k-tiles per group (sum = 80)
NCAST = 7          # f16 staging ring depth
NDUMMY = 6         # PE keep-warm matmuls bridging the AllGather valley
# psum bank chunks: layer-1 eviction + layer-2 column blocking
CHUNKS = ((0, 512), (512, 512), (1024, 256))
# the 30 pad columns per core sort to partitions 113..127 of tiles 8 and 9,
# so k-tiles with k%10 in (8,9) only have 113 real src rows
NPADROW = 113


def _rows(k):
    return NPADROW if k % 10 in (8, 9) else 128

_PROG_CACHE = {}


def _build_program(nocc=False, gsizes=GSIZES, ncast=NCAST, w1_ones=True,
                   ndummy=NDUMMY):
    import concourse.bacc as bacc
    import concourse.mybir as mybir
    from bass_rust import InstructionNameOrderedSet as _NameSet
    from concourse import tile

    f32 = mybir.dt.float32
    f16 = mybir.dt.float16
    u8 = mybir.dt.uint8
    grps = []
    _k0 = 0
    for _sz in gsizes:
        grps.append((_k0, _k0 + _sz))
        _k0 += _sz
    assert _k0 == KT
    maxg = max(k1 - k0 for k0, k1 in grps)

    nc = bacc.Bacc(
        "TRN2",
        target_bir_lowering=False,
        debug=False,
        enable_asserts=False,
        num_devices=1 if nocc else NCORES,
    )

    a = nc.dram_tensor("a", [KT, 128, SP], u8, kind="ExternalInput").ap()
    x0 = nc.dram_tensor("x0", [128, NPAD], f16, kind="ExternalInput").ap()
    # per-(bank chunk, partition) dequant scales; the host sorts dst
    # columns by quant range so each (chunk, partition) slot's 4 columns
    # share one scale -> the tanh eviction is 3 bank-wide activations
    csc = nc.dram_tensor("csc", [128, 3], f32, kind="ExternalInput").ap()
    # broadcast W1 row tiled x4 (only read when not w1_ones)
    w1b = nc.dram_tensor("w1b", [128, 512], f16, kind="ExternalInput").ap()
    out = nc.dram_tensor("out", [128, SP], f16, kind="ExternalOutput").ap()

    with tile.TileContext(nc) as tc:
        with (
            tc.tile_pool(name="xp", bufs=1) as xp,
            tc.tile_pool(name="a8p", bufs=1) as a8p,
            tc.tile_pool(name="fc", bufs=ncast) as fcp,
            tc.tile_pool(name="ps", bufs=1, space="PSUM") as ps,
            tc.tile_pool(name="dr", bufs=1, space="DRAM") as dr,
        ):
            # x0 is dead once layer 1 finishes; share one slot for both
            x0s = xp.tile([128, NPAD], f16, tag="xs")
            x1s = xp.tile([128, NPAD], f16, tag="xs")
            cscs = xp.tile([128, 3], f32, tag="cscs")
            w1s = xp.tile([128, 512], f16, tag="w1s")
            zl = xp.tile([128, 512], f16, tag="zl")
            warm = xp.tile([128, 1], f32, tag="warm")
            nc.scalar.dma_start(cscs[:], csc)
            if not w1_ones:
                nc.scalar.dma_start(w1s[:], w1b)
            nc.vector.memset(zl[:, 0:128], 0.0)
            nc.vector.memset(zl[:, 128:512], 0.0)
            # pre-load the ACT tanh table so the layer-1 eviction doesn't
            # pay the table load on the critical path
            nc.scalar.activation(
                warm[:], zl[:, 0:1], mybir.ActivationFunctionType.Tanh
            )

            agin = dr.tile([128, SP], f16)
            agout = dr.tile([NCORES * 128, SP], f16, addr_space="Shared")

            a8_tiles = {}

            def cast_group(gi, engines="vap", deps=None):
                """u8 -> f16 of resident group gi, split over the engines in
                `engines` (v=DVE, a=ACT in ~1us slices, p=POOL), shares
                proportional to their elementwise rates.  `deps` maps an
                engine letter to instruction names the slice must follow —
                used at the layer boundary so the scheduler cannot hoist
                casts ahead of the tanh -> AllGather chain."""
                k0, k1 = grps[gi]
                w = (k1 - k0) * SP
                a8 = a8_tiles[gi]
                fb = fcp.tile([128, maxg * SP], f16, tag="fc")
                rates = {"v": 4, "a": 4, "p": 3}
                tot = sum(rates[e] for e in engines)
                halves = deps.pop("halves", 1) if deps else 1
                bounds = [(w * h) // halves for h in range(halves + 1)]
                for h0, h1 in zip(bounds, bounds[1:]):
                  wh = h1 - h0
                  c0 = h0
                  for e in engines:
                    c1 = h1 if e == engines[-1] else c0 + (wh * rates[e]) // tot
                    insts = []
                    if e == "v":
                        insts.append(
                            nc.vector.tensor_copy(fb[:, c0:c1], a8[:, c0:c1])
                        )
                    elif e == "a":
                        # <=1.3k-elem slices so ACT never blocks the layer-1
                        # eviction chain behind a long copy
                        s0 = c0
                        while s0 < c1:
                            s1 = min(s0 + 1280, c1)
                            insts.append(
                                nc.scalar.copy(fb[:, s0:s1], a8[:, s0:s1])
                            )
                            s0 = s1
                    else:
                        insts.append(
                            nc.gpsimd.tensor_copy(fb[:, c0:c1], a8[:, c0:c1])
                        )
                    if deps and e in deps:
                        for inst in insts:
                            inst.ins.add_sync_dependencies_from(_NameSet(deps[e]))
                    c0 = c1
                return fb

            # ---- layer 1 (A-stationary; psum is [dst slot, feat]) ----
            # one psum tile per 2KiB bank: psum reads are dependency-tracked
            # whole-tile, so per-bank tiles let each bank's eviction start at
            # its own stop instead of after the layer's last matmul
            psum1 = []
            for ci, (c0, cn) in enumerate(CHUNKS):
                p1t = ps.tile([128, cn], f32, tag=f"acc1_{ci}", name=f"p1_{ci}")
                psum1.append(p1t)
            for ci, (c0, cn) in enumerate(CHUNKS):
                nc.tensor.matmul(
                    psum1[ci][:, 0:cn], zl[:, 0:128], zl[:, 0:cn],
                    start=True, stop=False,
                )
            # x0 for the first four groups rides ahead of their a8 loads so
            # the DMA queue can stay a couple of groups in front of the PE
            xlead = grps[3][1] * 128
            for gi, (k0, k1) in enumerate(grps):
                a8 = a8p.tile([128, (k1 - k0) * SP], u8, tag=f"a8_{gi}")
                a8_tiles[gi] = a8
                kb = {k0, k1}
                if 3 <= gi <= 6 and (k1 - k0) % 2 == 0:
                    kb.add((k0 + k1) // 2)
                for k in range(k0, k1 + 1):
                    if k % 10 in (8, 9) and k0 < k < k1 and _rows(k - 1) != \
                            NPADROW:
                        kb.add(k)
                    if k % 10 == 0 and k0 < k < k1 and _rows(k - 1) == \
                            NPADROW:
                        kb.add(k)
                kb = sorted(kb)
                for b0, b1 in zip(kb, kb[1:]):
                    nr = NPADROW if _rows(b0) == NPADROW else 128
                    nc.sync.dma_start(
                        a8[0:nr, (b0 - k0) * SP:(b1 - k0) * SP].rearrange(
                            "p (k j) -> p k j", k=b1 - b0
                        ),
                        a[b0:b1, 0:nr].rearrange("k p j -> p k j"),
                    )
                if gi == 0:
                    nc.sync.dma_start(x0s[:, 0:xlead], x0[:, 0:xlead])
                if gi >= 4:
                    nc.sync.dma_start(
                        x0s[:, k0 * 128:k1 * 128], x0[:, k0 * 128:k1 * 128]
                    )
            ng = len(grps)
            for oi, gi in enumerate(range(ng)):
                k0, k1 = grps[gi]
                fb = cast_group(gi, deps={"halves": 2}
                                if 3 <= gi <= 6 else None)
                if oi < ng - 1:
                    for k in range(k0, k1):
                        kk = k - k0
                        nr = _rows(k)
                        rhs = x0s[0:nr, k * 128:(k + 1) * 128]
                        for t in range(10):
                            ci, tt = (t // 4, t % 4)
                            nc.tensor.matmul(
                                psum1[ci][:, tt * 128:(tt + 1) * 128],
                                fb[0:nr, kk * SP + t * 128:
                                   kk * SP + (t + 1) * 128],
                                rhs,
                                start=False, stop=False,
                            )
                else:
                    # final group t-outer: each dst range finishes early so
                    # the tanh eviction overlaps the remaining matmuls
                    for t in range(10):
                        ci, tt = (t // 4, t % 4)
                        for k in range(k0, k1):
                            kk = k - k0
                            nr = _rows(k)
                            last_mm = nc.tensor.matmul(
                                psum1[ci][:, tt * 128:(tt + 1) * 128],
                                fb[0:nr, kk * SP + t * 128:
                                   kk * SP + (t + 1) * 128],
                                x0s[0:nr, k * 128:(k + 1) * 128],
                                start=False,
                                stop=(k == k1 - 1 and t in (3, 7, 9)),
                            )

            # pre-cast layer 2's first three groups (own-rank k-tiles,
            # no x1 dependency) on DVE while layer 1 drains, so layer 2's
            # matmuls start as soon as tanh chunk 0 lands
            with tc.high_priority():
                l2fb = {gi: cast_group(gi, "v") for gi in range(3)}

            # evict layer 1: x1 = tanh(cs_dst * psum1) [* W1] on ACT; DMA to
            # the AllGather bounce per psum bank so agin lands early.  The
            # whole tanh -> agin -> AllGather -> x1s chain is the only work
            # between the two PE-bound layers, so it runs at high priority
            # and its DMAs ride the otherwise-idle SP queue.
            agin_sb = xp.tile([128, SP], f16, tag="agin")
            # keep-warm matmuls: PE would otherwise idle across the AllGather
            # valley and restart cold (2.4x slower for the first 3us)
            psumd = ps.tile([128, 512], f32, tag="warmups")
            for _ in range(ndummy):
                dmm = nc.tensor.matmul(
                    psumd[:], zl[:, 0:128], zl[:, 0:512],
                    start=True, stop=True, skip_group_check=True,
                )
                # pin behind layer 1 so the scheduler cannot hoist the
                # warm-up matmuls to the (DMA-bound) start of the program
                dmm.ins.add_sync_dependencies_from(_NameSet([last_mm.ins.name]))
            tanh_last = None
            with tc.high_priority():
                for ci, (c0, cn) in enumerate(CHUNKS):
                    tanh_last = nc.scalar.activation(
                        agin_sb[:, c0:c0 + cn], psum1[ci][:, 0:cn],
                        mybir.ActivationFunctionType.Tanh,
                        scale=cscs[:, ci:ci + 1],
                    )
                    if not w1_ones:
                        nc.vector.tensor_mul(
                            agin_sb[:, c0:c0 + cn], agin_sb[:, c0:c0 + cn],
                            w1s[:, 0:cn]
                        )
                    nc.sync.dma_start(
                        agin[:, c0:c0 + cn], agin_sb[:, c0:c0 + cn]
                    )

                if nocc:
                    ag_inst = nc.sync.dma_start(agout[0:128, :], agin[:])
                else:
                    ag_inst = nc.gpsimd.collective_compute(
                        "AllGather",
                        mybir.AluOpType.bypass,
                        replica_groups=[list(range(NCORES))],
                        ins=[agin.opt()],
                        outs=[agout.opt()],
                    )
                # A's src row-blocks are rotated per core so block 0 is the
                # core's OWN rank: layer 2's first 10 k-tiles read agin_sb
                # directly (no AllGather round-trip), and block i (i>=1) is
                # rank (pid+i)%8, fetched from agout at a register-computed
                # offset.
                agdep = _NameSet([ag_inst.ins.name])
                pid = nc.sync.partition_id()
                x1dmas = []
                for i in range(1, NCORES):
                    boff = ((pid + i) % NCORES) * (128 * SP)
                    if i == 1:
                        # lead slice so k-tile 10 can start while the rest
                        # of the block is in flight
                        dsrc = agout[0:128, 0:384].copy()
                        dsrc.offset = boff
                        x1dmas.append(
                            nc.sync.dma_start(x1s[:, SP:SP + 384], dsrc)
                        )
                        dsrc = agout[0:128, 384:SP].copy()
                        dsrc.offset = boff + 384
                        x1dmas.append(
                            nc.sync.dma_start(
                                x1s[:, SP + 384:2 * SP], dsrc
                            )
                        )
                    else:
                        dsrc = agout[0:128, :].copy()
                        dsrc.offset = boff
                        x1dmas.append(
                            nc.sync.dma_start(
                                x1s[:, i * SP:(i + 1) * SP], dsrc
                            )
                        )
                # DRAM->SBUF reads of the collective output are not tracked
                # as data deps in the single-core twin; pin them so the
                # scheduler cannot float them ahead of the agin writes
                for d in x1dmas:
                    d.ins.add_sync_dependencies_from(agdep)

            # ---- layer 2 (X-stationary; psum is [feat, dst]) ----
            # All of A is already resident as u8; only the casts re-run.
            # The first ring of casts has no x1 dependency, so it completes
            # during the AllGather and PE starts as soon as rank 0 lands.
            psum2 = []
            for ci, (c0, cn) in enumerate(CHUNKS):
                p2t = ps.tile([128, cn], f32, tag=f"acc2_{ci}", name=f"p2_{ci}")
                psum2.append(p2t)
            ob = xp.tile([128, SP], f16, tag="ob")

            def lhsT_of(k):
                # row-block 0 is the core's own rank: its activations are
                # already on-chip in agin_sb (same [dst slot, feat] layout)
                nr = _rows(k)
                if k < 10:
                    return agin_sb[0:nr, k * 128:(k + 1) * 128]
                return x1s[0:nr, k * 128:(k + 1) * 128]

            first = True
            tdep = [tanh_last.ins.name]
            adep = [ag_inst.ins.name]
            for gi, (k0, k1) in enumerate(grps):
                # keep ACT free for the tanh chain and POOL free for the
                # AllGather issue while the boundary groups pre-cast on DVE;
                # ACT/POOL rejoin once their part of the chain retires
                if gi < 3:
                    fb = l2fb[gi]
                elif gi < 5:
                    fb = cast_group(gi, "va", deps={"a": tdep})
                elif gi < 9:
                    fb = cast_group(gi, "vap", deps={"a": tdep, "p": adep})
                else:
                    fb = cast_group(gi)
                last_grp = gi == len(grps) - 1
                if not last_grp:
                    for k in range(k0, k1):
                        kk = k - k0
                        lhsT = lhsT_of(k)
                        nr = _rows(k)
                        for ci, (c0, cn) in enumerate(CHUNKS):
                            nc.tensor.matmul(
                                psum2[ci][:, 0:cn],
                                lhsT,
                                fb[0:nr, kk * SP + c0: kk * SP + c0 + cn],
                                start=first, stop=False,
                            )
                        first = False
                else:
                    # final group: bank-outer with per-bank stops; ALL
                    # evictions are created after the matmuls (psum reads
                    # are tracked whole-tile, so an earlier-created read
                    # would falsely serialize the later banks' matmuls)
                    for ci in (2, 0, 1):
                        c0, cn = CHUNKS[ci]
                        for k in range(k0, k1):
                            kk = k - k0
                            nc.tensor.matmul(
                                psum2[ci][:, 0:cn],
                                lhsT_of(k),
                                fb[0:_rows(k), kk * SP + c0:
                                   kk * SP + c0 + cn],
                                start=False, stop=(k == k1 - 1),
                            )
                    # GPSIMD cannot read PSUM on HW: evict banks on DVE/ACT
                    dq_eng = (nc.sync, nc.sync, nc.scalar)
                    with tc.high_priority():
                        for ci in (2, 0, 1):
                            c0, cn = CHUNKS[ci]
                            if ci == 1:
                                nc.scalar.copy(
                                    ob[:, c0:c0 + cn], psum2[ci][:, 0:cn]
                                )
                            else:
                                nc.vector.tensor_copy(
                                    ob[:, c0:c0 + cn], psum2[ci][:, 0:cn]
                                )
                            dq_eng[ci].dma_start(
                                out[:, c0:c0 + cn], ob[:, c0:c0 + cn]
                            )

    nc.compile()
    return nc


def get_program(nocc=False, gsizes=GSIZES, ncast=NCAST, w1_ones=True,
                ndummy=NDUMMY):
    key = ("nc", nocc, tuple(gsizes), ncast, w1_ones, ndummy)
    if key not in _PROG_CACHE:
        _PROG_CACHE[key] = _build_program(nocc, gsizes, ncast, w1_ones,
                                          ndummy)
    return _PROG_CACHE[key]


def _slot_order():
    """Slot s = t*128 + p (tile t in 0..9, partition p) listed in quant-sort
    order: chunks of 4 (banks 0/1) or 2 (bank 2) consecutive sorted columns
    share one (chunk, partition) slot group, hence one dequant scale."""
    slots = np.empty(SP, np.int64)
    i = 0
    for ci, (tile0, ntile) in enumerate(((0, 4), (4, 4), (8, 2))):
        for p in range(128):
            for ti in range(ntile):
                slots[i] = (tile0 + ti) * 128 + p
                i += 1
    assert i == SP
    return slots


_SLOTS = _slot_order()


def _core_perm(colmax_ext):
    """perm[s] = original local dst column (or >=S for pad) in slot s, with
    columns sorted by quant range so slot groups share a scale."""
    order = np.argsort(-colmax_ext, kind="stable")  # [SP] sorted col ids
    perm = np.empty(SP, np.int64)
    perm[_SLOTS] = order
    return perm


def build_in_maps(x, src, dst, vals, W):
    """Host-side prep: dense A^T shard (u8 quantized, 4 sorted columns per
    scale group) + x0, both in the per-core permuted slot order."""
    import scipy.sparse as sp

    x = np.asarray(x, np.float32)
    src = np.asarray(src, np.int64)
    dst = np.asarray(dst, np.int64)
    vals = np.asarray(vals, np.float32)
    W = np.asarray(W, np.float32)

    # A[dst, src] = sum of vals  ->  we build AT[src, dst]
    AT = sp.coo_matrix((vals, (src, dst)), shape=(N, N)).toarray()

    # per-core column permutations (dst side of A, src rows of A, x rows)
    perms = []
    steps = []
    cscs = []
    for c in range(NCORES):
        ATc = AT[:, c * S:(c + 1) * S]  # [N, S] float32
        colmax_ext = np.full(SP, -1.0, np.float32)
        colmax_ext[:S] = ATc.max(axis=0)
        perm = _core_perm(colmax_ext)
        # group scale = max colmax over each slot group (same (chunk, p))
        cm_slot = np.maximum(colmax_ext[perm], 1e-9)  # [SP] by slot
        step_slot = np.empty(SP, np.float32)
        csc = np.empty((128, 3), np.float32)
        for ci, (tile0, ntile) in enumerate(((0, 4), (4, 4), (8, 2))):
            t_sl = slice(tile0 * 128, (tile0 + ntile) * 128)
            cm = cm_slot[t_sl].reshape(ntile, 128)    # [ntile, p]
            gmax = cm.max(axis=0) / 255.0             # [p]
            csc[:, ci] = gmax
            step_slot[t_sl] = np.tile(gmax[None, :], (ntile, 1)).reshape(-1)
        perms.append(perm)
        steps.append(step_slot)
        cscs.append(np.ascontiguousarray(csc))

    # per-core src slot -> node mapping: row-block i of core c is rank
    # (c+i)%8 (own rank first, so layer 2 starts from on-chip activations),
    # permuted within the block by that rank's own column permutation
    node2s, valid2s = [], []
    for c in range(NCORES):
        node2 = np.empty(NPAD, np.int64)
        valid2 = np.empty(NPAD, bool)
        for i in range(NCORES):
            r = (c + i) % NCORES
            pr = perms[r]
            valid = pr < S
            node2[i * SP:(i + 1) * SP] = np.where(valid, r * S + pr, 0)
            valid2[i * SP:(i + 1) * SP] = valid
        node2s.append(node2)
        valid2s.append(valid2)

    xw = x * W[0][None, :]

    w1brow = np.ascontiguousarray(
        np.tile(W[1][None, :], (128, 4))
    ).astype(np.float16)

    in_maps = []
    for c in range(NCORES):
        node2, valid2 = node2s[c], valid2s[c]
        x0p = np.zeros((NPAD, D), np.float32)
        x0p[valid2] = xw[node2[valid2]]
        x0h = np.ascontiguousarray(
            x0p.reshape(KT, 128, D).transpose(1, 0, 2).reshape(128, KT * D)
        ).astype(np.float16)
        ATc = AT[:, c * S:(c + 1) * S]  # [N, S] float32
        perm = perms[c]
        valid = perm < S
        ATs = np.zeros((N, SP), np.float32)
        ATs[:, valid] = ATc[:, perm[valid]]           # columns in slot order
        Aq = np.clip(np.rint(ATs / steps[c][None, :]), 0, 255).astype(
            np.uint8
        )
        Ap = np.zeros((NPAD, SP), Aq.dtype)
        Ap[valid2] = Aq[node2[valid2]]                # rows in slot order
        a3 = np.ascontiguousarray(Ap.reshape(KT, 128, SP))
        in_maps.append(
            {
                "a": a3,
                "x0": x0h,
                "csc": cscs[c],
                "w1b": w1brow,
            }
        )
    return in_maps, (steps, perms)


def assemble_output(results, aux):
    steps, perms = aux
    outs = []
    for c in range(NCORES):
        ot = np.asarray(results[c]["out"], np.float32)  # [128, SP] feat-major
        ot = ot * steps[c][None, :]  # per-dst dequant (layer-2)
        perm = perms[c]
        valid = perm < S
        o = np.zeros((S, 128), np.float32)
        o[perm[valid]] = ot[:, valid].T             # un-permute dst slots
        outs.append(o)
    return np.ascontiguousarray(np.concatenate(outs, axis=0))


def kernel(x, src, dst, vals, W):
    from concourse import bass_utils

    w1_ones = bool(np.all(np.asarray(W)[1] == 1.0))
    nc = get_program(w1_ones=w1_ones)
    in_maps, steps = build_in_maps(x, src, dst, vals, W)
    # The axon terminal can wedge when a different program was loaded
    # earlier in its lifetime; after the crash the terminal restarts and a
    # retry succeeds.  Back off progressively to ride out the restart.
    import time as _time

    last_err = None
    for sleep_s in (10.0, 30.0, 60.0, 0.0):
        try:
            res = bass_utils.run_bass_kernel_spmd(
                nc, in_maps, core_ids=list(range(NCORES))
            )
            return assemble_output(res.results, steps)
        except Exception as e:  # noqa: BLE001
            last_err = e
            _time.sleep(sleep_s)
    raise last_err


# revision 37
# speedup vs baseline: 1.0582x; 1.0062x over previous
"""GCN diag-encoder (2-layer SpMM) on 8 Trainium2 NeuronCores.

Strategy: the sparse adjacency (640K edges over 10K nodes, ~0.64% dense) is
materialized as a dense A^T matrix on the host; each per-layer
  out[dst] = sum_e vals[e] * x[src[e]]        (segment-sum SpMM)
becomes dense TensorEngine matmuls.  Each core owns a 1250-wide dst slice of
A^T (padded to 1280, uint8-quantized per dst column).

v3: A^T is DMA'd ONCE as raw uint8 (half the DMA bytes of a u8->f16
cast-DMA, which is charged at the f16 destination size) and stays resident
in SBUF (100KB/partition).  The u8->f16 conversion runs on-chip, split
across the three otherwise-idle compute engines (DVE / Activation / GpSimd)
into a rotating ring of f16 staging tiles that feed the PE.  Both layers
re-cast from the same resident u8 copy, so layer 2 needs no A traffic at
all.  This turns layer 1 from DMA-bound (~93us) into PE-bound (~45us) and
removes layer 2's 29us f16 re-stream.

Layer 1 runs A-stationary — matmul(out=psum[dst,feat], lhsT=AT_tile[src,dst],
rhs=x_tile[src,feat]) — so the layer-1 output is node-major: the eviction is
a fused tanh+dequant-scale pass on the scalar engine straight into the
AllGather bounce.  The host sorts each core's dst columns by quantization
range and packs 4 similar columns per (psum bank, partition) slot, so the
dequant scale is per-partition within a bank and the whole eviction is 3
bank-wide activations (full per-column accuracy at bank-chunk cost).  Each
psum bank is its own tile (psum reads are dependency-tracked whole-tile, so
per-bank tiles let each bank's eviction start at its own stop) and is
seeded by one full-width start=True zero matmul.  Layer 2 runs X-stationary
— matmul(out=psum[feat,dst], lhsT=x1_tile[src,feat], rhs=AT_tile[src,dst]);
its dequant scale and the final un-permute are applied on the host.

Src row-blocks are rotated per core so block 0 is the core's OWN rank:
layer 2's first 10 k-tiles read the tanh output agin_sb directly from SBUF
(no AllGather round-trip), hiding most of the collective latency behind
real work; the other 7 blocks are fetched from the AllGather output at
register-computed offsets ((partition_id+i)%8).  A few zl-by-zl keep-warm
matmuls stop the PE from dropping out of its max p-state across the
remaining gap.  W0 is folded into x on the host; W1 is skipped on device
when it is all-ones (torch init), else applied via a broadcast multiply.
"""

import numpy as np

N = 10000          # nodes
D = 128            # feature dim
NCORES = 8
S = 1250           # dst nodes per core
SP = 1280          # padded dst per core (10 tiles of 128)
KT = 80            # contraction k-tiles (padded src rows = 10240)
NPAD = KT * 128    # 10240
GSIZES = (2, 2) + (4,) * 19      # k-tiles per group (sum = 80)
NCAST = 7          # f16 staging ring depth
NDUMMY = 6         # PE keep-warm matmuls bridging the AllGather valley
# psum bank chunks: layer-1 eviction + layer-2 column blocking
CHUNKS = ((0, 512), (512, 512), (1024, 256))
# the 30 pad columns per core sort to partitions 113..127 of tiles 8 and 9,
# so k-tiles with k%10 in (8,9) only have 113 real src rows
NPADROW = 113


def _rows(k):
    return NPADROW if k % 10 in (8, 9) else 128

_PROG_CACHE = {}


def _build_program(nocc=False, gsizes=GSIZES, ncast=NCAST, w1_ones=True,
                   ndummy=NDUMMY):
    import concourse.bacc as bacc
    import concourse.mybir as mybir
    from bass_rust import InstructionNameOrderedSet as _NameSet
    from concourse import tile

    f32 = mybir.dt.float32
    f16 = mybir.dt.float16
    u8 = mybir.dt.uint8
    grps = []
    _k0 = 0
    for _sz in gsizes:
        grps.append((_k0, _k0 + _sz))
        _k0 += _sz
    assert _k0 == KT
    maxg = max(k1 - k0 for k0, k1 in grps)

    nc = bacc.Bacc(
        "TRN2",
        target_bir_lowering=False,
        debug=False,
        enable_asserts=False,
        num_devices=1 if nocc else NCORES,
    )

    a = nc.dram_tensor("a", [KT, 128, SP], u8, kind="ExternalInput").ap()
    x0 = nc.dram_tensor("x0", [128, NPAD], f16, kind="ExternalInput").ap()
    # per-(bank chunk, partition) dequant scales; the host sorts dst
    # columns by quant range so each (chunk, partition) slot's 4 columns
    # share one scale -> the tanh eviction is 3 bank-wide activations
    csc = nc.dram_tensor("csc", [128, 3], f32, kind="ExternalInput").ap()
    # broadcast W1 row tiled x4 (only read when not w1_ones)
    w1b = nc.dram_tensor("w1b", [128, 512], f16, kind="ExternalInput").ap()
    out = nc.dram_tensor("out", [128, SP], f16, kind="ExternalOutput").ap()

    with tile.TileContext(nc) as tc:
        with (
            tc.tile_pool(name="xp", bufs=1) as xp,
            tc.tile_pool(name="a8p", bufs=1) as a8p,
            tc.tile_pool(name="fc", bufs=ncast) as fcp,
            tc.tile_pool(name="ps", bufs=1, space="PSUM") as ps,
            tc.tile_pool(name="dr", bufs=1, space="DRAM") as dr,
        ):
            # x0 is dead once layer 1 finishes; share one slot for both
            x0s = xp.tile([128, NPAD], f16, tag="xs")
            x1s = xp.tile([128, NPAD], f16, tag="xs")
            cscs = xp.tile([128, 3], f32, tag="cscs")
            w1s = xp.tile([128, 512], f16, tag="w1s")
            zl = xp.tile([128, 512], f16, tag="zl")
            warm = xp.tile([128, 1], f32, tag="warm")
            nc.scalar.dma_start(cscs[:], csc)
            if not w1_ones:
                nc.scalar.dma_start(w1s[:], w1b)
            nc.vector.memset(zl[:, 0:128], 0.0)
            nc.vector.memset(zl[:, 128:512], 0.0)
            # pre-load the ACT tanh table so the layer-1 eviction doesn't
            # pay the table load on the critical path
            nc.scalar.activation(
                warm[:], zl[:, 0:1], mybir.ActivationFunctionType.Tanh
            )

            # the AllGather is split in two chunked collectives so the
            # first remote block's head (cols 0-512) lands right after tanh
            # chunk 0, shrinking the boundary valley
            agin1 = dr.tile([128, 512], f16)
            agin2 = dr.tile([128, SP - 512], f16)
            agout1 = dr.tile([NCORES * 128, 512], f16, addr_space="Shared")
            agout2 = dr.tile([NCORES * 128, SP - 512], f16,
                             addr_space="Shared")

            a8_tiles = {}

            def cast_group(gi, engines="vap", deps=None):
                """u8 -> f16 of resident group gi, split over the engines in
                `engines` (v=DVE, a=ACT in ~1us slices, p=POOL), shares
                proportional to their elementwise rates.  `deps` maps an
                engine letter to instruction names the slice must follow —
                used at the layer boundary so the scheduler cannot hoist
                casts ahead of the tanh -> AllGather chain."""
                k0, k1 = grps[gi]
                w = (k1 - k0) * SP
                a8 = a8_tiles[gi]
                fb = fcp.tile([128, maxg * SP], f16, tag="fc")
                rates = {"v": 4, "a": 4, "p": 3}
                tot = sum(rates[e] for e in engines)
                halves = deps.pop("halves", 1) if deps else 1
                bounds = [(w * h) // halves for h in range(halves + 1)]
                for h0, h1 in zip(bounds, bounds[1:]):
                  wh = h1 - h0
                  c0 = h0
                  for e in engines:
                    c1 = h1 if e == engines[-1] else c0 + (wh * rates[e]) // tot
                    insts = []
                    if e == "v":
                        insts.append(
                            nc.vector.tensor_copy(fb[:, c0:c1], a8[:, c0:c1])
                        )
                    elif e == "a":
                        # <=1.3k-elem slices so ACT never blocks the layer-1
                        # eviction chain behind a long copy
                        s0 = c0
                        while s0 < c1:
                            s1 = min(s0 + 1280, c1)
                            insts.append(
                                nc.scalar.copy(fb[:, s0:s1], a8[:, s0:s1])
                            )
                            s0 = s1
                    else:
                        insts.append(
                            nc.gpsimd.tensor_copy(fb[:, c0:c1], a8[:, c0:c1])
                        )
                    if deps and e in deps:
                        for inst in insts:
                            inst.ins.add_sync_dependencies_from(_NameSet(deps[e]))
                    c0 = c1
                return fb

            # ---- layer 1 (A-stationary; psum is [dst slot, feat]) ----
            # one psum tile per 2KiB bank: psum reads are dependency-tracked
            # whole-tile, so per-bank tiles let each bank's eviction start at
            # its own stop instead of after the layer's last matmul
            psum1 = []
            for ci, (c0, cn) in enumerate(CHUNKS):
                p1t = ps.tile([128, cn], f32, tag=f"acc1_{ci}", name=f"p1_{ci}")
                psum1.append(p1t)
            for ci, (c0, cn) in enumerate(CHUNKS):
                nc.tensor.matmul(
                    psum1[ci][:, 0:cn], zl[:, 0:128], zl[:, 0:cn],
                    start=True, stop=False,
                )
            # x0 for the first four groups rides ahead of their a8 loads so
            # the DMA queue can stay a couple of groups in front of the PE
            xlead = grps[3][1] * 128
            for gi, (k0, k1) in enumerate(grps):
                a8 = a8p.tile([128, (k1 - k0) * SP], u8, tag=f"a8_{gi}")
                a8_tiles[gi] = a8
                kb = {k0, k1}
                if 3 <= gi <= 6 and (k1 - k0) % 2 == 0:
                    kb.add((k0 + k1) // 2)
                for k in range(k0, k1 + 1):
                    if k % 10 in (8, 9) and k0 < k < k1 and _rows(k - 1) != \
                            NPADROW:
                        kb.add(k)
                    if k % 10 == 0 and k0 < k < k1 and _rows(k - 1) == \
                            NPADROW:
                        kb.add(k)
                kb = sorted(kb)
                for b0, b1 in zip(kb, kb[1:]):
                    nr = NPADROW if _rows(b0) == NPADROW else 128
                    nc.sync.dma_start(
                        a8[0:nr, (b0 - k0) * SP:(b1 - k0) * SP].rearrange(
                            "p (k j) -> p k j", k=b1 - b0
                        ),
                        a[b0:b1, 0:nr].rearrange("k p j -> p k j"),
                    )
                if gi == 0:
                    nc.sync.dma_start(x0s[:, 0:xlead], x0[:, 0:xlead])
                if gi >= 4:
                    nc.sync.dma_start(
                        x0s[:, k0 * 128:k1 * 128], x0[:, k0 * 128:k1 * 128]
                    )
            ng = len(grps)
            for oi, gi in enumerate(range(ng)):
                k0, k1 = grps[gi]
                fb = cast_group(gi, deps={"halves": 2}
                                if 3 <= gi <= 6 else None)
                if oi < ng - 1:
                    for k in range(k0, k1):
                        kk = k - k0
                        nr = _rows(k)
                        rhs = x0s[0:nr, k * 128:(k + 1) * 128]
                        for t in range(10):
                            ci, tt = (t // 4, t % 4)
                            nc.tensor.matmul(
                                psum1[ci][:, tt * 128:(tt + 1) * 128],
                                fb[0:nr, kk * SP + t * 128:
                                   kk * SP + (t + 1) * 128],
                                rhs,
                                start=False, stop=False,
                            )
                else:
                    # final group t-outer: each dst range finishes early so
                    # the tanh eviction overlaps the remaining matmuls
                    for t in range(10):
                        ci, tt = (t // 4, t % 4)
                        for k in range(k0, k1):
                            kk = k - k0
                            nr = _rows(k)
                            last_mm = nc.tensor.matmul(
                                psum1[ci][:, tt * 128:(tt + 1) * 128],
                                fb[0:nr, kk * SP + t * 128:
                                   kk * SP + (t + 1) * 128],
                                x0s[0:nr, k * 128:(k + 1) * 128],
                                start=False,
                                stop=(k == k1 - 1 and t in (3, 7, 9)),
                            )

            # pre-cast layer 2's first three groups (own-rank k-tiles,
            # no x1 dependency) on DVE while layer 1 drains, so layer 2's
            # matmuls start as soon as tanh chunk 0 lands
            with tc.high_priority():
                l2fb = {gi: cast_group(gi, "v") for gi in range(3)}

            # evict layer 1: x1 = tanh(cs_dst * psum1) [* W1] on ACT; DMA to
            # the AllGather bounce per psum bank so agin lands early.  The
            # whole tanh -> agin -> AllGather -> x1s chain is the only work
            # between the two PE-bound layers, so it runs at high priority
            # and its DMAs ride the otherwise-idle SP queue.
            agin_sb = xp.tile([128, SP], f16, tag="agin")
            # keep-warm matmuls: PE would otherwise idle across the AllGather
            # valley and restart cold (2.4x slower for the first 3us)
            psumd = ps.tile([128, 512], f32, tag="warmups")
            for _ in range(ndummy):
                dmm = nc.tensor.matmul(
                    psumd[:], zl[:, 0:128], zl[:, 0:512],
                    start=True, stop=True, skip_group_check=True,
                )
                # pin behind layer 1 so the scheduler cannot hoist the
                # warm-up matmuls to the (DMA-bound) start of the program
                dmm.ins.add_sync_dependencies_from(_NameSet([last_mm.ins.name]))
            tanh_last = None
            with tc.high_priority():
                ag1 = None
                agin2_dmas = []
                for ci, (c0, cn) in enumerate(CHUNKS):
                    tanh_last = nc.scalar.activation(
                        agin_sb[:, c0:c0 + cn], psum1[ci][:, 0:cn],
                        mybir.ActivationFunctionType.Tanh,
                        scale=cscs[:, ci:ci + 1],
                    )
                    if not w1_ones:
                        nc.vector.tensor_mul(
                            agin_sb[:, c0:c0 + cn], agin_sb[:, c0:c0 + cn],
                            w1s[:, 0:cn]
                        )
                    if ci == 0:
                        ad = nc.sync.dma_start(agin1[:], agin_sb[:, 0:512])
                        if nocc:
                            ag1 = nc.sync.dma_start(
                                agout1[0:128, :], agin1[:])
                        else:
                            ag1 = nc.gpsimd.collective_compute(
                                "AllGather",
                                mybir.AluOpType.bypass,
                                replica_groups=[list(range(NCORES))],
                                ins=[agin1.opt()],
                                outs=[agout1.opt()],
                            )
                        ag1.ins.add_sync_dependencies_from(
                            _NameSet([ad.ins.name]))
                    else:
                        ad = nc.sync.dma_start(
                            agin2[:, c0 - 512:c0 - 512 + cn],
                            agin_sb[:, c0:c0 + cn])
                        agin2_dmas.append(ad.ins.name)
                if nocc:
                    ag_inst = nc.sync.dma_start(agout2[0:128, :], agin2[:])
                else:
                    ag_inst = nc.gpsimd.collective_compute(
                        "AllGather",
                        mybir.AluOpType.bypass,
                        replica_groups=[list(range(NCORES))],
                        ins=[agin2.opt()],
                        outs=[agout2.opt()],
                    )
                ag_inst.ins.add_sync_dependencies_from(
                    _NameSet(agin2_dmas + [ag1.ins.name]))
                # A's src row-blocks are rotated per core so block 0 is the
                # core's OWN rank: layer 2's first 10 k-tiles read agin_sb
                # directly (no AllGather round-trip), and block i (i>=1) is
                # rank (pid+i)%8, fetched from agout at a register-computed
                # offset.
                ag1dep = _NameSet([ag1.ins.name])
                ag2dep = _NameSet([ag_inst.ins.name])
                pid = nc.sync.partition_id()
                for i in range(1, NCORES):
                    rank = (pid + i) % NCORES
                    dsrc = agout1[0:128, :].copy()
                    dsrc.offset = rank * (128 * 512)
                    xa = nc.sync.dma_start(
                        x1s[:, i * SP:i * SP + 512], dsrc)
                    # DRAM->SBUF reads of the collective output are not
                    # tracked as data deps; pin them behind the collectives
                    xa.ins.add_sync_dependencies_from(ag1dep)
                    dsrc = agout2[0:128, :].copy()
                    dsrc.offset = rank * (128 * (SP - 512))
                    xb = nc.sync.dma_start(
                        x1s[:, i * SP + 512:(i + 1) * SP], dsrc)
                    xb.ins.add_sync_dependencies_from(ag2dep)

            # ---- layer 2 (X-stationary; psum is [feat, dst]) ----
            # All of A is already resident as u8; only the casts re-run.
            # The first ring of casts has no x1 dependency, so it completes
            # during the AllGather and PE starts as soon as rank 0 lands.
            psum2 = []
            for ci, (c0, cn) in enumerate(CHUNKS):
                p2t = ps.tile([128, cn], f32, tag=f"acc2_{ci}", name=f"p2_{ci}")
                psum2.append(p2t)
            ob = xp.tile([128, SP], f16, tag="ob")

            def lhsT_of(k):
                # row-block 0 is the core's own rank: its activations are
                # already on-chip in agin_sb (same [dst slot, feat] layout)
                nr = _rows(k)
                if k < 10:
                    return agin_sb[0:nr, k * 128:(k + 1) * 128]
                return x1s[0:nr, k * 128:(k + 1) * 128]

            first = True
            tdep = [tanh_last.ins.name]
            adep = [ag_inst.ins.name]
            for gi, (k0, k1) in enumerate(grps):
                # keep ACT free for the tanh chain and POOL free for the
                # AllGather issue while the boundary groups pre-cast on DVE;
                # ACT/POOL rejoin once their part of the chain retires
                if gi < 3:
                    fb = l2fb[gi]
                elif gi < 5:
                    fb = cast_group(gi, "va", deps={"a": tdep})
                elif gi < 9:
                    fb = cast_group(gi, "vap", deps={"a": tdep, "p": adep})
                else:
                    fb = cast_group(gi)
                last_grp = gi == len(grps) - 1
                if not last_grp:
                    for k in range(k0, k1):
                        kk = k - k0
                        lhsT = lhsT_of(k)
                        nr = _rows(k)
                        for ci, (c0, cn) in enumerate(CHUNKS):
                            nc.tensor.matmul(
                                psum2[ci][:, 0:cn],
                                lhsT,
                                fb[0:nr, kk * SP + c0: kk * SP + c0 + cn],
                                start=first, stop=False,
                            )
                        first = False
                else:
                    # final group: bank-outer with per-bank stops; ALL
                    # evictions are created after the matmuls (psum reads
                    # are tracked whole-tile, so an earlier-created read
                    # would falsely serialize the later banks' matmuls)
                    for ci in (2, 0, 1):
                        c0, cn = CHUNKS[ci]
                        for k in range(k0, k1):
                            kk = k - k0
                            nc.tensor.matmul(
                                psum2[ci][:, 0:cn],
                                lhsT_of(k),
                                fb[0:_rows(k), kk * SP + c0:
                                   kk * SP + c0 + cn],
                                start=False, stop=(k == k1 - 1),
                            )
                    # GPSIMD cannot read PSUM on HW: evict banks on DVE/ACT
                    dq_eng = (nc.sync, nc.sync, nc.scalar)
                    with tc.high_priority():
                        for ci in (2, 0, 1):
                            c0, cn = CHUNKS[ci]
                            if ci == 1:
                                nc.scalar.copy(
                                    ob[:, c0:c0 + cn], psum2[ci][:, 0:cn]
                                )
                            else:
                                nc.vector.tensor_copy(
                                    ob[:, c0:c0 + cn], psum2[ci][:, 0:cn]
                                )
                            dq_eng[ci].dma_start(
                                out[:, c0:c0 + cn], ob[:, c0:c0 + cn]
                            )

    nc.compile()
    return nc


def get_program(nocc=False, gsizes=GSIZES, ncast=NCAST, w1_ones=True,
                ndummy=NDUMMY):
    key = ("nc", nocc, tuple(gsizes), ncast, w1_ones, ndummy)
    if key not in _PROG_CACHE:
        _PROG_CACHE[key] = _build_program(nocc, gsizes, ncast, w1_ones,
                                          ndummy)
    return _PROG_CACHE[key]


def _slot_order():
    """Slot s = t*128 + p (tile t in 0..9, partition p) listed in quant-sort
    order: chunks of 4 (banks 0/1) or 2 (bank 2) consecutive sorted columns
    share one (chunk, partition) slot group, hence one dequant scale."""
    slots = np.empty(SP, np.int64)
    i = 0
    for ci, (tile0, ntile) in enumerate(((0, 4), (4, 4), (8, 2))):
        for p in range(128):
            for ti in range(ntile):
                slots[i] = (tile0 + ti) * 128 + p
                i += 1
    assert i == SP
    return slots


_SLOTS = _slot_order()


def _core_perm(colmax_ext):
    """perm[s] = original local dst column (or >=S for pad) in slot s, with
    columns sorted by quant range so slot groups share a scale."""
    order = np.argsort(-colmax_ext, kind="stable")  # [SP] sorted col ids
    perm = np.empty(SP, np.int64)
    perm[_SLOTS] = order
    return perm


def build_in_maps(x, src, dst, vals, W):
    """Host-side prep: dense A^T shard (u8 quantized, 4 sorted columns per
    scale group) + x0, both in the per-core permuted slot order."""
    import scipy.sparse as sp

    x = np.asarray(x, np.float32)
    src = np.asarray(src, np.int64)
    dst = np.asarray(dst, np.int64)
    vals = np.asarray(vals, np.float32)
    W = np.asarray(W, np.float32)

    # A[dst, src] = sum of vals  ->  we build AT[src, dst]
    AT = sp.coo_matrix((vals, (src, dst)), shape=(N, N)).toarray()

    # per-core column permutations (dst side of A, src rows of A, x rows)
    perms = []
    steps = []
    cscs = []
    for c in range(NCORES):
        ATc = AT[:, c * S:(c + 1) * S]  # [N, S] float32
        colmax_ext = np.full(SP, -1.0, np.float32)
        colmax_ext[:S] = ATc.max(axis=0)
        perm = _core_perm(colmax_ext)
        # group scale = max colmax over each slot group (same (chunk, p))
        cm_slot = np.maximum(colmax_ext[perm], 1e-9)  # [SP] by slot
        step_slot = np.empty(SP, np.float32)
        csc = np.empty((128, 3), np.float32)
        for ci, (tile0, ntile) in enumerate(((0, 4), (4, 4), (8, 2))):
            t_sl = slice(tile0 * 128, (tile0 + ntile) * 128)
            cm = cm_slot[t_sl].reshape(ntile, 128)    # [ntile, p]
            gmax = cm.max(axis=0) / 255.0             # [p]
            csc[:, ci] = gmax
            step_slot[t_sl] = np.tile(gmax[None, :], (ntile, 1)).reshape(-1)
        perms.append(perm)
        steps.append(step_slot)
        cscs.append(np.ascontiguousarray(csc))

    # per-core src slot -> node mapping: row-block i of core c is rank
    # (c+i)%8 (own rank first, so layer 2 starts from on-chip activations),
    # permuted within the block by that rank's own column permutation
    node2s, valid2s = [], []
    for c in range(NCORES):
        node2 = np.empty(NPAD, np.int64)
        valid2 = np.empty(NPAD, bool)
        for i in range(NCORES):
            r = (c + i) % NCORES
            pr = perms[r]
            valid = pr < S
            node2[i * SP:(i + 1) * SP] = np.where(valid, r * S + pr, 0)
            valid2[i * SP:(i + 1) * SP] = valid
        node2s.append(node2)
        valid2s.append(valid2)

    xw = x * W[0][None, :]

    w1brow = np.ascontiguousarray(
        np.tile(W[1][None, :], (128, 4))
    ).astype(np.float16)

    in_maps = []
    for c in range(NCORES):
        node2, valid2 = node2s[c], valid2s[c]
        x0p = np.zeros((NPAD, D), np.float32)
        x0p[valid2] = xw[node2[valid2]]
        x0h = np.ascontiguousarray(
            x0p.reshape(KT, 128, D).transpose(1, 0, 2).reshape(128, KT * D)
        ).astype(np.float16)
        ATc = AT[:, c * S:(c + 1) * S]  # [N, S] float32
        perm = perms[c]
        valid = perm < S
        ATs = np.zeros((N, SP), np.float32)
        ATs[:, valid] = ATc[:, perm[valid]]           # columns in slot order
        Aq = np.clip(np.rint(ATs / steps[c][None, :]), 0, 255).astype(
            np.uint8
        )
        Ap = np.zeros((NPAD, SP), Aq.dtype)
        Ap[valid2] = Aq[node2[valid2]]                # rows in slot order
        a3 = np.ascontiguousarray(Ap.reshape(KT, 128, SP))
        in_maps.append(
            {
                "a": a3,
                "x0": x0h,
                "csc": cscs[c],
                "w1b": w1brow,
            }
        )
    return in_maps, (steps, perms)


def assemble_output(results, aux):
    steps, perms = aux
    outs = []
    for c in range(NCORES):
        ot = np.asarray(results[c]["out"], np.float32)  # [128, SP] feat-major
        ot = ot * steps[c][None, :]  # per-dst dequant (layer-2)
        perm = perms[c]
        valid = perm < S
        o = np.zeros((S, 128), np.float32)
        o[perm[valid]] = ot[:, valid].T             # un-permute dst slots
        outs.append(o)
    return np.ascontiguousarray(np.concatenate(outs, axis=0))


def kernel(x, src, dst, vals, W):
    from concourse import bass_utils

    w1_ones = bool(np.all(np.asarray(W)[1] == 1.0))
    nc = get_program(w1_ones=w1_ones)
    in_maps, steps = build_in_maps(x, src, dst, vals, W)
    # The axon terminal can wedge when a different program was loaded
    # earlier in its lifetime; after the crash the terminal restarts and a
    # retry succeeds.  Back off progressively to ride out the restart.
    import time as _time

    last_err = None
    for sleep_s in (10.0, 30.0, 60.0, 0.0):
        try:
            res = bass_utils.run_bass_kernel_spmd(
                nc, in_maps, core_ids=list(range(NCORES))
            )
            return assemble_output(res.results, steps)
        except Exception as e:  # noqa: BLE001
            last_err = e
            _time.sleep(sleep_s)
    raise last_err


# revision 38
# speedup vs baseline: 1.0716x; 1.0127x over previous
"""GCN diag-encoder (2-layer SpMM) on 8 Trainium2 NeuronCores.

Strategy: the sparse adjacency (640K edges over 10K nodes, ~0.64% dense) is
materialized as a dense A^T matrix on the host; each per-layer
  out[dst] = sum_e vals[e] * x[src[e]]        (segment-sum SpMM)
becomes dense TensorEngine matmuls.  Each core owns a 1250-wide dst slice of
A^T (padded to 1280, uint8-quantized per dst column).

v3: A^T is DMA'd ONCE as raw uint8 (half the DMA bytes of a u8->f16
cast-DMA, which is charged at the f16 destination size) and stays resident
in SBUF (100KB/partition).  The u8->f16 conversion runs on-chip, split
across the three otherwise-idle compute engines (DVE / Activation / GpSimd)
into a rotating ring of f16 staging tiles that feed the PE.  Both layers
re-cast from the same resident u8 copy, so layer 2 needs no A traffic at
all.  This turns layer 1 from DMA-bound (~93us) into PE-bound (~45us) and
removes layer 2's 29us f16 re-stream.

Layer 1 runs A-stationary — matmul(out=psum[dst,feat], lhsT=AT_tile[src,dst],
rhs=x_tile[src,feat]) — so the layer-1 output is node-major: the eviction is
a fused tanh+dequant-scale pass on the scalar engine straight into the
AllGather bounce.  The host sorts each core's dst columns by quantization
range and packs 4 similar columns per (psum bank, partition) slot, so the
dequant scale is per-partition within a bank and the whole eviction is 3
bank-wide activations (full per-column accuracy at bank-chunk cost).  Each
psum bank is its own tile (psum reads are dependency-tracked whole-tile, so
per-bank tiles let each bank's eviction start at its own stop) and is
seeded by one full-width start=True zero matmul.  Layer 2 runs X-stationary
— matmul(out=psum[feat,dst], lhsT=x1_tile[src,feat], rhs=AT_tile[src,dst]);
its dequant scale and the final un-permute are applied on the host.

Src row-blocks are rotated per core so block 0 is the core's OWN rank:
layer 2's first 10 k-tiles read the tanh output agin_sb directly from SBUF
(no AllGather round-trip), hiding most of the collective latency behind
real work; the other 7 blocks are fetched from the AllGather output at
register-computed offsets ((partition_id+i)%8).  A few zl-by-zl keep-warm
matmuls stop the PE from dropping out of its max p-state across the
remaining gap.  W0 is folded into x on the host; W1 is skipped on device
when it is all-ones (torch init), else applied via a broadcast multiply.
"""

import numpy as np

N = 10000          # nodes
D = 128            # feature dim
NCORES = 8
S = 1250           # dst nodes per core
SP = 1280          # padded dst per core (10 tiles of 128)
KT = 80            # contraction k-tiles (padded src rows = 10240)
NPAD = KT * 128    # 10240
GSIZES = (2, 2) + (4,) * 19      # k-tiles per group (sum = 80)
NCAST = 7          # f16 staging ring depth
NDUMMY = 6         # PE keep-warm matmuls bridging the AllGather valley
# psum bank chunks: layer-1 eviction + layer-2 column blocking
CHUNKS = ((0, 512), (512, 512), (1024, 256))
# the 30 pad columns per core sort to partitions 113..127 of tiles 8 and 9,
# so k-tiles with k%10 in (8,9) only have 113 real src rows
NPADROW = 113


def _rows(k):
    return NPADROW if k % 10 in (8, 9) else 128

_PROG_CACHE = {}


def _build_program(nocc=False, gsizes=GSIZES, ncast=NCAST, w1_ones=True,
                   ndummy=NDUMMY):
    import concourse.bacc as bacc
    import concourse.mybir as mybir
    from bass_rust import InstructionNameOrderedSet as _NameSet
    from concourse import tile

    f32 = mybir.dt.float32
    f16 = mybir.dt.float16
    u8 = mybir.dt.uint8
    grps = []
    _k0 = 0
    for _sz in gsizes:
        grps.append((_k0, _k0 + _sz))
        _k0 += _sz
    assert _k0 == KT
    maxg = max(k1 - k0 for k0, k1 in grps)

    nc = bacc.Bacc(
        "TRN2",
        target_bir_lowering=False,
        debug=False,
        enable_asserts=False,
        num_devices=1 if nocc else NCORES,
    )

    a = nc.dram_tensor("a", [KT, 128, SP], u8, kind="ExternalInput").ap()
    x0 = nc.dram_tensor("x0", [128, NPAD], f16, kind="ExternalInput").ap()
    # per-(bank chunk, partition) dequant scales; the host sorts dst
    # columns by quant range so each (chunk, partition) slot's 4 columns
    # share one scale -> the tanh eviction is 3 bank-wide activations
    csc = nc.dram_tensor("csc", [128, 3], f32, kind="ExternalInput").ap()
    # broadcast W1 row tiled x4 (only read when not w1_ones)
    w1b = nc.dram_tensor("w1b", [128, 512], f16, kind="ExternalInput").ap()
    out = nc.dram_tensor("out", [128, SP], f16, kind="ExternalOutput").ap()

    with tile.TileContext(nc) as tc:
        with (
            tc.tile_pool(name="xp", bufs=1) as xp,
            tc.tile_pool(name="a8p", bufs=1) as a8p,
            tc.tile_pool(name="fc", bufs=ncast) as fcp,
            tc.tile_pool(name="ps", bufs=1, space="PSUM") as ps,
            tc.tile_pool(name="dr", bufs=1, space="DRAM") as dr,
        ):
            # x0 is dead once layer 1 finishes; share one slot for both
            x0s = xp.tile([128, NPAD], f16, tag="xs")
            x1s = xp.tile([128, NPAD], f16, tag="xs")
            cscs = xp.tile([128, 3], f32, tag="cscs")
            w1s = xp.tile([128, 512], f16, tag="w1s")
            zl = xp.tile([128, 512], f16, tag="zl")
            warm = xp.tile([128, 1], f32, tag="warm")
            nc.scalar.dma_start(cscs[:], csc)
            if not w1_ones:
                nc.scalar.dma_start(w1s[:], w1b)
            nc.vector.memset(zl[:, 0:128], 0.0)
            nc.vector.memset(zl[:, 128:512], 0.0)
            # pre-load the ACT tanh table so the layer-1 eviction doesn't
            # pay the table load on the critical path
            nc.scalar.activation(
                warm[:], zl[:, 0:1], mybir.ActivationFunctionType.Tanh
            )

            # the AllGather is split in two chunked collectives so the
            # first remote block's head (cols 0-512) lands right after tanh
            # chunk 0, shrinking the boundary valley
            agin1 = dr.tile([128, 512], f16)
            agin2 = dr.tile([128, SP - 512], f16)
            agout1 = dr.tile([NCORES * 128, 512], f16, addr_space="Shared")
            agout2 = dr.tile([NCORES * 128, SP - 512], f16,
                             addr_space="Shared")

            a8_tiles = {}

            def cast_group(gi, engines="vap", deps=None):
                """u8 -> f16 of resident group gi, split over the engines in
                `engines` (v=DVE, a=ACT in ~1us slices, p=POOL), shares
                proportional to their elementwise rates.  `deps` maps an
                engine letter to instruction names the slice must follow —
                used at the layer boundary so the scheduler cannot hoist
                casts ahead of the tanh -> AllGather chain."""
                k0, k1 = grps[gi]
                w = (k1 - k0) * SP
                a8 = a8_tiles[gi]
                fb = fcp.tile([128, maxg * SP], f16, tag="fc")
                rates = {"v": 4, "a": 4, "p": 3}
                tot = sum(rates[e] for e in engines)
                halves = deps.pop("halves", 1) if deps else 1
                bounds = [(w * h) // halves for h in range(halves + 1)]
                for h0, h1 in zip(bounds, bounds[1:]):
                  wh = h1 - h0
                  c0 = h0
                  for e in engines:
                    c1 = h1 if e == engines[-1] else c0 + (wh * rates[e]) // tot
                    insts = []
                    if e == "v":
                        insts.append(
                            nc.vector.tensor_copy(fb[:, c0:c1], a8[:, c0:c1])
                        )
                    elif e == "a":
                        # <=1.3k-elem slices so ACT never blocks the layer-1
                        # eviction chain behind a long copy
                        s0 = c0
                        while s0 < c1:
                            s1 = min(s0 + 1280, c1)
                            insts.append(
                                nc.scalar.copy(fb[:, s0:s1], a8[:, s0:s1])
                            )
                            s0 = s1
                    else:
                        insts.append(
                            nc.gpsimd.tensor_copy(fb[:, c0:c1], a8[:, c0:c1])
                        )
                    if deps and e in deps:
                        for inst in insts:
                            inst.ins.add_sync_dependencies_from(_NameSet(deps[e]))
                    c0 = c1
                return fb

            # ---- layer 1 (A-stationary; psum is [dst slot, feat]) ----
            # one psum tile per 2KiB bank: psum reads are dependency-tracked
            # whole-tile, so per-bank tiles let each bank's eviction start at
            # its own stop instead of after the layer's last matmul
            psum1 = []
            for ci, (c0, cn) in enumerate(CHUNKS):
                p1t = ps.tile([128, cn], f32, tag=f"acc1_{ci}", name=f"p1_{ci}")
                psum1.append(p1t)
            for ci, (c0, cn) in enumerate(CHUNKS):
                nc.tensor.matmul(
                    psum1[ci][:, 0:cn], zl[:, 0:128], zl[:, 0:cn],
                    start=True, stop=False,
                )
            # x0 for the first four groups rides ahead of their a8 loads so
            # the DMA queue can stay a couple of groups in front of the PE
            xlead = grps[3][1] * 128
            for gi, (k0, k1) in enumerate(grps):
                a8 = a8p.tile([128, (k1 - k0) * SP], u8, tag=f"a8_{gi}")
                a8_tiles[gi] = a8
                kb = {k0, k1}
                if 3 <= gi <= 6 and (k1 - k0) % 2 == 0:
                    kb.add((k0 + k1) // 2)
                for k in range(k0, k1 + 1):
                    if k % 10 in (8, 9) and k0 < k < k1 and _rows(k - 1) != \
                            NPADROW:
                        kb.add(k)
                    if k % 10 == 0 and k0 < k < k1 and _rows(k - 1) == \
                            NPADROW:
                        kb.add(k)
                kb = sorted(kb)
                for b0, b1 in zip(kb, kb[1:]):
                    nr = NPADROW if _rows(b0) == NPADROW else 128
                    nc.sync.dma_start(
                        a8[0:nr, (b0 - k0) * SP:(b1 - k0) * SP].rearrange(
                            "p (k j) -> p k j", k=b1 - b0
                        ),
                        a[b0:b1, 0:nr].rearrange("k p j -> p k j"),
                    )
                if gi == 0:
                    nc.sync.dma_start(x0s[:, 0:xlead], x0[:, 0:xlead])
                if gi >= 4:
                    nc.sync.dma_start(
                        x0s[:, k0 * 128:k1 * 128], x0[:, k0 * 128:k1 * 128]
                    )
            ng = len(grps)
            for oi, gi in enumerate(range(ng)):
                k0, k1 = grps[gi]
                fb = cast_group(gi, deps={"halves": 2}
                                if 3 <= gi <= 6 else None)
                if oi < ng - 1:
                    for k in range(k0, k1):
                        kk = k - k0
                        nr = _rows(k)
                        rhs = x0s[0:nr, k * 128:(k + 1) * 128]
                        for t in range(10):
                            ci, tt = (t // 4, t % 4)
                            nc.tensor.matmul(
                                psum1[ci][:, tt * 128:(tt + 1) * 128],
                                fb[0:nr, kk * SP + t * 128:
                                   kk * SP + (t + 1) * 128],
                                rhs,
                                start=False, stop=False,
                            )
                else:
                    # final group t-outer: each dst range finishes early so
                    # the tanh eviction overlaps the remaining matmuls
                    for t in range(10):
                        ci, tt = (t // 4, t % 4)
                        for k in range(k0, k1):
                            kk = k - k0
                            nr = _rows(k)
                            last_mm = nc.tensor.matmul(
                                psum1[ci][:, tt * 128:(tt + 1) * 128],
                                fb[0:nr, kk * SP + t * 128:
                                   kk * SP + (t + 1) * 128],
                                x0s[0:nr, k * 128:(k + 1) * 128],
                                start=False,
                                stop=(k == k1 - 1 and t in (3, 7, 9)),
                            )

            # pre-cast layer 2's first three groups (own-rank k-tiles,
            # no x1 dependency) on DVE while layer 1 drains, so layer 2's
            # matmuls start as soon as tanh chunk 0 lands
            with tc.high_priority():
                l2fb = {gi: cast_group(gi, "v") for gi in range(3)}

            # evict layer 1: x1 = tanh(cs_dst * psum1) [* W1] on ACT; DMA to
            # the AllGather bounce per psum bank so agin lands early.  The
            # whole tanh -> agin -> AllGather -> x1s chain is the only work
            # between the two PE-bound layers, so it runs at high priority
            # and its DMAs ride the otherwise-idle SP queue.
            agin_sb = xp.tile([128, SP], f16, tag="agin")
            # keep-warm matmuls: PE would otherwise idle across the AllGather
            # valley and restart cold (2.4x slower for the first 3us)
            psumd = ps.tile([128, 512], f32, tag="warmups")
            for _ in range(ndummy):
                dmm = nc.tensor.matmul(
                    psumd[:], zl[:, 0:128], zl[:, 0:512],
                    start=True, stop=True, skip_group_check=True,
                )
                # pin behind layer 1 so the scheduler cannot hoist the
                # warm-up matmuls to the (DMA-bound) start of the program
                dmm.ins.add_sync_dependencies_from(_NameSet([last_mm.ins.name]))
            tanh_last = None
            with tc.high_priority():
                ag1 = None
                agin2_dmas = []
                for ci, (c0, cn) in enumerate(CHUNKS):
                    tanh_last = nc.scalar.activation(
                        agin_sb[:, c0:c0 + cn], psum1[ci][:, 0:cn],
                        mybir.ActivationFunctionType.Tanh,
                        scale=cscs[:, ci:ci + 1],
                    )
                    if not w1_ones:
                        nc.vector.tensor_mul(
                            agin_sb[:, c0:c0 + cn], agin_sb[:, c0:c0 + cn],
                            w1s[:, 0:cn]
                        )
                    if ci == 0:
                        ad = nc.sync.dma_start(agin1[:], agin_sb[:, 0:512])
                        if nocc:
                            ag1 = nc.sync.dma_start(
                                agout1[0:128, :], agin1[:])
                        else:
                            ag1 = nc.gpsimd.collective_compute(
                                "AllGather",
                                mybir.AluOpType.bypass,
                                replica_groups=[list(range(NCORES))],
                                ins=[agin1.opt()],
                                outs=[agout1.opt()],
                            )
                        ag1.ins.add_sync_dependencies_from(
                            _NameSet([ad.ins.name]))
                    else:
                        ad = nc.sync.dma_start(
                            agin2[:, c0 - 512:c0 - 512 + cn],
                            agin_sb[:, c0:c0 + cn])
                        agin2_dmas.append(ad.ins.name)
                if nocc:
                    ag_inst = nc.sync.dma_start(agout2[0:128, :], agin2[:])
                else:
                    ag_inst = nc.gpsimd.collective_compute(
                        "AllGather",
                        mybir.AluOpType.bypass,
                        replica_groups=[list(range(NCORES))],
                        ins=[agin2.opt()],
                        outs=[agout2.opt()],
                    )
                ag_inst.ins.add_sync_dependencies_from(
                    _NameSet(agin2_dmas))
                # A's src row-blocks are rotated per core so block 0 is the
                # core's OWN rank: layer 2's first 10 k-tiles read agin_sb
                # directly (no AllGather round-trip), and block i (i>=1) is
                # rank (pid+i)%8, fetched from agout at a register-computed
                # offset.
                ag1dep = _NameSet([ag1.ins.name])
                ag2dep = _NameSet([ag_inst.ins.name])
                pid = nc.sync.partition_id()
                for i in range(1, NCORES):
                    rank = (pid + i) % NCORES
                    dsrc = agout1[0:128, :].copy()
                    dsrc.offset = rank * (128 * 512)
                    xa = nc.sync.dma_start(
                        x1s[:, i * SP:i * SP + 512], dsrc)
                    # DRAM->SBUF reads of the collective output are not
                    # tracked as data deps; pin them behind the collectives
                    xa.ins.add_sync_dependencies_from(ag1dep)
                    dsrc = agout2[0:128, :].copy()
                    dsrc.offset = rank * (128 * (SP - 512))
                    xb = nc.sync.dma_start(
                        x1s[:, i * SP + 512:(i + 1) * SP], dsrc)
                    xb.ins.add_sync_dependencies_from(ag2dep)

            # ---- layer 2 (X-stationary; psum is [feat, dst]) ----
            # All of A is already resident as u8; only the casts re-run.
            # The first ring of casts has no x1 dependency, so it completes
            # during the AllGather and PE starts as soon as rank 0 lands.
            psum2 = []
            for ci, (c0, cn) in enumerate(CHUNKS):
                p2t = ps.tile([128, cn], f32, tag=f"acc2_{ci}", name=f"p2_{ci}")
                psum2.append(p2t)
            ob = xp.tile([128, SP], f16, tag="ob")

            def lhsT_of(k):
                # row-block 0 is the core's own rank: its activations are
                # already on-chip in agin_sb (same [dst slot, feat] layout)
                nr = _rows(k)
                if k < 10:
                    return agin_sb[0:nr, k * 128:(k + 1) * 128]
                return x1s[0:nr, k * 128:(k + 1) * 128]

            first = True
            tdep = [tanh_last.ins.name]
            adep = [ag_inst.ins.name]
            for gi, (k0, k1) in enumerate(grps):
                # keep ACT free for the tanh chain and POOL free for the
                # AllGather issue while the boundary groups pre-cast on DVE;
                # ACT/POOL rejoin once their part of the chain retires
                if gi < 3:
                    fb = l2fb[gi]
                elif gi < 5:
                    fb = cast_group(gi, "va", deps={"a": tdep})
                elif gi < 9:
                    fb = cast_group(gi, "vap", deps={"a": tdep, "p": adep})
                else:
                    fb = cast_group(gi)
                last_grp = gi == len(grps) - 1
                if not last_grp:
                    for k in range(k0, k1):
                        kk = k - k0
                        lhsT = lhsT_of(k)
                        nr = _rows(k)
                        for ci, (c0, cn) in enumerate(CHUNKS):
                            nc.tensor.matmul(
                                psum2[ci][:, 0:cn],
                                lhsT,
                                fb[0:nr, kk * SP + c0: kk * SP + c0 + cn],
                                start=first, stop=False,
                            )
                        first = False
                else:
                    # final group: bank-outer with per-bank stops; ALL
                    # evictions are created after the matmuls (psum reads
                    # are tracked whole-tile, so an earlier-created read
                    # would falsely serialize the later banks' matmuls)
                    for ci in (2, 0, 1):
                        c0, cn = CHUNKS[ci]
                        for k in range(k0, k1):
                            kk = k - k0
                            nc.tensor.matmul(
                                psum2[ci][:, 0:cn],
                                lhsT_of(k),
                                fb[0:_rows(k), kk * SP + c0:
                                   kk * SP + c0 + cn],
                                start=False, stop=(k == k1 - 1),
                            )
                    # GPSIMD cannot read PSUM on HW: evict banks on DVE/ACT
                    dq_eng = (nc.sync, nc.sync, nc.scalar)
                    with tc.high_priority():
                        for ci in (2, 0, 1):
                            c0, cn = CHUNKS[ci]
                            if ci == 1:
                                nc.scalar.copy(
                                    ob[:, c0:c0 + cn], psum2[ci][:, 0:cn]
                                )
                            else:
                                nc.vector.tensor_copy(
                                    ob[:, c0:c0 + cn], psum2[ci][:, 0:cn]
                                )
                            dq_eng[ci].dma_start(
                                out[:, c0:c0 + cn], ob[:, c0:c0 + cn]
                            )

    nc.compile()
    return nc


def get_program(nocc=False, gsizes=GSIZES, ncast=NCAST, w1_ones=True,
                ndummy=NDUMMY):
    key = ("nc", nocc, tuple(gsizes), ncast, w1_ones, ndummy)
    if key not in _PROG_CACHE:
        _PROG_CACHE[key] = _build_program(nocc, gsizes, ncast, w1_ones,
                                          ndummy)
    return _PROG_CACHE[key]


def _slot_order():
    """Slot s = t*128 + p (tile t in 0..9, partition p) listed in quant-sort
    order: chunks of 4 (banks 0/1) or 2 (bank 2) consecutive sorted columns
    share one (chunk, partition) slot group, hence one dequant scale."""
    slots = np.empty(SP, np.int64)
    i = 0
    for ci, (tile0, ntile) in enumerate(((0, 4), (4, 4), (8, 2))):
        for p in range(128):
            for ti in range(ntile):
                slots[i] = (tile0 + ti) * 128 + p
                i += 1
    assert i == SP
    return slots


_SLOTS = _slot_order()


def _core_perm(colmax_ext):
    """perm[s] = original local dst column (or >=S for pad) in slot s, with
    columns sorted by quant range so slot groups share a scale."""
    order = np.argsort(-colmax_ext, kind="stable")  # [SP] sorted col ids
    perm = np.empty(SP, np.int64)
    perm[_SLOTS] = order
    return perm


def build_in_maps(x, src, dst, vals, W):
    """Host-side prep: dense A^T shard (u8 quantized, 4 sorted columns per
    scale group) + x0, both in the per-core permuted slot order."""
    import scipy.sparse as sp

    x = np.asarray(x, np.float32)
    src = np.asarray(src, np.int64)
    dst = np.asarray(dst, np.int64)
    vals = np.asarray(vals, np.float32)
    W = np.asarray(W, np.float32)

    # A[dst, src] = sum of vals  ->  we build AT[src, dst]
    AT = sp.coo_matrix((vals, (src, dst)), shape=(N, N)).toarray()

    # per-core column permutations (dst side of A, src rows of A, x rows)
    perms = []
    steps = []
    cscs = []
    for c in range(NCORES):
        ATc = AT[:, c * S:(c + 1) * S]  # [N, S] float32
        colmax_ext = np.full(SP, -1.0, np.float32)
        colmax_ext[:S] = ATc.max(axis=0)
        perm = _core_perm(colmax_ext)
        # group scale = max colmax over each slot group (same (chunk, p))
        cm_slot = np.maximum(colmax_ext[perm], 1e-9)  # [SP] by slot
        step_slot = np.empty(SP, np.float32)
        csc = np.empty((128, 3), np.float32)
        for ci, (tile0, ntile) in enumerate(((0, 4), (4, 4), (8, 2))):
            t_sl = slice(tile0 * 128, (tile0 + ntile) * 128)
            cm = cm_slot[t_sl].reshape(ntile, 128)    # [ntile, p]
            gmax = cm.max(axis=0) / 255.0             # [p]
            csc[:, ci] = gmax
            step_slot[t_sl] = np.tile(gmax[None, :], (ntile, 1)).reshape(-1)
        perms.append(perm)
        steps.append(step_slot)
        cscs.append(np.ascontiguousarray(csc))

    # per-core src slot -> node mapping: row-block i of core c is rank
    # (c+i)%8 (own rank first, so layer 2 starts from on-chip activations),
    # permuted within the block by that rank's own column permutation
    node2s, valid2s = [], []
    for c in range(NCORES):
        node2 = np.empty(NPAD, np.int64)
        valid2 = np.empty(NPAD, bool)
        for i in range(NCORES):
            r = (c + i) % NCORES
            pr = perms[r]
            valid = pr < S
            node2[i * SP:(i + 1) * SP] = np.where(valid, r * S + pr, 0)
            valid2[i * SP:(i + 1) * SP] = valid
        node2s.append(node2)
        valid2s.append(valid2)

    xw = x * W[0][None, :]

    w1brow = np.ascontiguousarray(
        np.tile(W[1][None, :], (128, 4))
    ).astype(np.float16)

    in_maps = []
    for c in range(NCORES):
        node2, valid2 = node2s[c], valid2s[c]
        x0p = np.zeros((NPAD, D), np.float32)
        x0p[valid2] = xw[node2[valid2]]
        x0h = np.ascontiguousarray(
            x0p.reshape(KT, 128, D).transpose(1, 0, 2).reshape(128, KT * D)
        ).astype(np.float16)
        ATc = AT[:, c * S:(c + 1) * S]  # [N, S] float32
        perm = perms[c]
        valid = perm < S
        ATs = np.zeros((N, SP), np.float32)
        ATs[:, valid] = ATc[:, perm[valid]]           # columns in slot order
        Aq = np.clip(np.rint(ATs / steps[c][None, :]), 0, 255).astype(
            np.uint8
        )
        Ap = np.zeros((NPAD, SP), Aq.dtype)
        Ap[valid2] = Aq[node2[valid2]]                # rows in slot order
        a3 = np.ascontiguousarray(Ap.reshape(KT, 128, SP))
        in_maps.append(
            {
                "a": a3,
                "x0": x0h,
                "csc": cscs[c],
                "w1b": w1brow,
            }
        )
    return in_maps, (steps, perms)


def assemble_output(results, aux):
    steps, perms = aux
    outs = []
    for c in range(NCORES):
        ot = np.asarray(results[c]["out"], np.float32)  # [128, SP] feat-major
        ot = ot * steps[c][None, :]  # per-dst dequant (layer-2)
        perm = perms[c]
        valid = perm < S
        o = np.zeros((S, 128), np.float32)
        o[perm[valid]] = ot[:, valid].T             # un-permute dst slots
        outs.append(o)
    return np.ascontiguousarray(np.concatenate(outs, axis=0))


def kernel(x, src, dst, vals, W):
    from concourse import bass_utils

    w1_ones = bool(np.all(np.asarray(W)[1] == 1.0))
    nc = get_program(w1_ones=w1_ones)
    in_maps, steps = build_in_maps(x, src, dst, vals, W)
    # The axon terminal can wedge when a different program was loaded
    # earlier in its lifetime; after the crash the terminal restarts and a
    # retry succeeds.  Back off progressively to ride out the restart.
    import time as _time

    last_err = None
    for sleep_s in (10.0, 30.0, 60.0, 0.0):
        try:
            res = bass_utils.run_bass_kernel_spmd(
                nc, in_maps, core_ids=list(range(NCORES))
            )
            return assemble_output(res.results, steps)
        except Exception as e:  # noqa: BLE001
            last_err = e
            _time.sleep(sleep_s)
    raise last_err


# revision 39
# speedup vs baseline: 1.0763x; 1.0044x over previous
"""GCN diag-encoder (2-layer SpMM) on 8 Trainium2 NeuronCores.

Strategy: the sparse adjacency (640K edges over 10K nodes, ~0.64% dense) is
materialized as a dense A^T matrix on the host; each per-layer
  out[dst] = sum_e vals[e] * x[src[e]]        (segment-sum SpMM)
becomes dense TensorEngine matmuls.  Each core owns a 1250-wide dst slice of
A^T (padded to 1280, uint8-quantized per dst column).

v3: A^T is DMA'd ONCE as raw uint8 (half the DMA bytes of a u8->f16
cast-DMA, which is charged at the f16 destination size) and stays resident
in SBUF (100KB/partition).  The u8->f16 conversion runs on-chip, split
across the three otherwise-idle compute engines (DVE / Activation / GpSimd)
into a rotating ring of f16 staging tiles that feed the PE.  Both layers
re-cast from the same resident u8 copy, so layer 2 needs no A traffic at
all.  This turns layer 1 from DMA-bound (~93us) into PE-bound (~45us) and
removes layer 2's 29us f16 re-stream.

Layer 1 runs A-stationary — matmul(out=psum[dst,feat], lhsT=AT_tile[src,dst],
rhs=x_tile[src,feat]) — so the layer-1 output is node-major: the eviction is
a fused tanh+dequant-scale pass on the scalar engine straight into the
AllGather bounce.  The host sorts each core's dst columns by quantization
range and packs 4 similar columns per (psum bank, partition) slot, so the
dequant scale is per-partition within a bank and the whole eviction is 3
bank-wide activations (full per-column accuracy at bank-chunk cost).  Each
psum bank is its own tile (psum reads are dependency-tracked whole-tile, so
per-bank tiles let each bank's eviction start at its own stop) and is
seeded by one full-width start=True zero matmul.  Layer 2 runs X-stationary
— matmul(out=psum[feat,dst], lhsT=x1_tile[src,feat], rhs=AT_tile[src,dst]);
its dequant scale and the final un-permute are applied on the host.

Src row-blocks are rotated per core so block 0 is the core's OWN rank:
layer 2's first 10 k-tiles read the tanh output agin_sb directly from SBUF
(no AllGather round-trip), hiding most of the collective latency behind
real work; the other 7 blocks are fetched from the AllGather output at
register-computed offsets ((partition_id+i)%8).  A few zl-by-zl keep-warm
matmuls stop the PE from dropping out of its max p-state across the
remaining gap.  W0 is folded into x on the host; W1 is skipped on device
when it is all-ones (torch init), else applied via a broadcast multiply.
"""

import numpy as np

N = 10000          # nodes
D = 128            # feature dim
NCORES = 8
S = 1250           # dst nodes per core
SP = 1280          # padded dst per core (10 tiles of 128)
KT = 80            # contraction k-tiles (padded src rows = 10240)
NPAD = KT * 128    # 10240
GSIZES = (2, 2) + (4,) * 19      # k-tiles per group (sum = 80)
NCAST = 7          # f16 staging ring depth
NDUMMY = 6         # PE keep-warm matmuls bridging the AllGather valley
# psum bank chunks: layer-1 eviction + layer-2 column blocking
CHUNKS = ((0, 512), (512, 512), (1024, 256))
# layer-2 moving spans: the last 15 dst slots (1265-1279) are always pad
# columns (the 30 pads sort last, half landing on tile 9's tail), so the
# layer-2 matmuls skip them outright
CH2 = ((0, 512), (512, 512), (1024, 241))
# the 30 pad columns per core sort to partitions 113..127 of tiles 8 and 9,
# so k-tiles with k%10 in (8,9) only have 113 real src rows
NPADROW = 113


def _rows(k):
    return NPADROW if k % 10 in (8, 9) else 128

_PROG_CACHE = {}


def _build_program(nocc=False, gsizes=GSIZES, ncast=NCAST, w1_ones=True,
                   ndummy=NDUMMY):
    import concourse.bacc as bacc
    import concourse.mybir as mybir
    from bass_rust import InstructionNameOrderedSet as _NameSet
    from concourse import tile

    f32 = mybir.dt.float32
    f16 = mybir.dt.float16
    u8 = mybir.dt.uint8
    grps = []
    _k0 = 0
    for _sz in gsizes:
        grps.append((_k0, _k0 + _sz))
        _k0 += _sz
    assert _k0 == KT
    maxg = max(k1 - k0 for k0, k1 in grps)

    nc = bacc.Bacc(
        "TRN2",
        target_bir_lowering=False,
        debug=False,
        enable_asserts=False,
        num_devices=1 if nocc else NCORES,
    )

    a = nc.dram_tensor("a", [KT, 128, SP], u8, kind="ExternalInput").ap()
    x0 = nc.dram_tensor("x0", [128, NPAD], f16, kind="ExternalInput").ap()
    # per-(bank chunk, partition) dequant scales; the host sorts dst
    # columns by quant range so each (chunk, partition) slot's 4 columns
    # share one scale -> the tanh eviction is 3 bank-wide activations
    csc = nc.dram_tensor("csc", [128, 3], f32, kind="ExternalInput").ap()
    # broadcast W1 row tiled x4 (only read when not w1_ones)
    w1b = nc.dram_tensor("w1b", [128, 512], f16, kind="ExternalInput").ap()
    out = nc.dram_tensor("out", [128, SP], f16, kind="ExternalOutput").ap()

    with tile.TileContext(nc) as tc:
        with (
            tc.tile_pool(name="xp", bufs=1) as xp,
            tc.tile_pool(name="a8p", bufs=1) as a8p,
            tc.tile_pool(name="fc", bufs=ncast) as fcp,
            tc.tile_pool(name="ps", bufs=1, space="PSUM") as ps,
            tc.tile_pool(name="dr", bufs=1, space="DRAM") as dr,
        ):
            # x0 is dead once layer 1 finishes; share one slot for both
            x0s = xp.tile([128, NPAD], f16, tag="xs")
            x1s = xp.tile([128, NPAD], f16, tag="xs")
            cscs = xp.tile([128, 3], f32, tag="cscs")
            w1s = xp.tile([128, 512], f16, tag="w1s")
            zl = xp.tile([128, 512], f16, tag="zl")
            warm = xp.tile([128, 1], f32, tag="warm")
            nc.scalar.dma_start(cscs[:], csc)
            if not w1_ones:
                nc.scalar.dma_start(w1s[:], w1b)
            nc.vector.memset(zl[:, 0:128], 0.0)
            nc.vector.memset(zl[:, 128:512], 0.0)
            # pre-load the ACT tanh table so the layer-1 eviction doesn't
            # pay the table load on the critical path
            nc.scalar.activation(
                warm[:], zl[:, 0:1], mybir.ActivationFunctionType.Tanh
            )

            # the AllGather is split in two chunked collectives so the
            # first remote block's head (cols 0-512) lands right after tanh
            # chunk 0, shrinking the boundary valley
            agin1 = dr.tile([128, 512], f16)
            agin2 = dr.tile([128, SP - 512], f16)
            agout1 = dr.tile([NCORES * 128, 512], f16, addr_space="Shared")
            agout2 = dr.tile([NCORES * 128, SP - 512], f16,
                             addr_space="Shared")

            a8_tiles = {}

            def cast_group(gi, engines="vap", deps=None):
                """u8 -> f16 of resident group gi, split over the engines in
                `engines` (v=DVE, a=ACT in ~1us slices, p=POOL), shares
                proportional to their elementwise rates.  `deps` maps an
                engine letter to instruction names the slice must follow —
                used at the layer boundary so the scheduler cannot hoist
                casts ahead of the tanh -> AllGather chain."""
                k0, k1 = grps[gi]
                w = (k1 - k0) * SP
                a8 = a8_tiles[gi]
                fb = fcp.tile([128, maxg * SP], f16, tag="fc")
                rates = {"v": 4, "a": 4, "p": 3}
                tot = sum(rates[e] for e in engines)
                halves = deps.pop("halves", 1) if deps else 1
                bounds = [(w * h) // halves for h in range(halves + 1)]
                for h0, h1 in zip(bounds, bounds[1:]):
                  wh = h1 - h0
                  c0 = h0
                  for e in engines:
                    c1 = h1 if e == engines[-1] else c0 + (wh * rates[e]) // tot
                    insts = []
                    if e == "v":
                        insts.append(
                            nc.vector.tensor_copy(fb[:, c0:c1], a8[:, c0:c1])
                        )
                    elif e == "a":
                        # <=1.3k-elem slices so ACT never blocks the layer-1
                        # eviction chain behind a long copy
                        s0 = c0
                        while s0 < c1:
                            s1 = min(s0 + 1280, c1)
                            insts.append(
                                nc.scalar.copy(fb[:, s0:s1], a8[:, s0:s1])
                            )
                            s0 = s1
                    else:
                        insts.append(
                            nc.gpsimd.tensor_copy(fb[:, c0:c1], a8[:, c0:c1])
                        )
                    if deps and e in deps:
                        for inst in insts:
                            inst.ins.add_sync_dependencies_from(_NameSet(deps[e]))
                    c0 = c1
                return fb

            # ---- layer 1 (A-stationary; psum is [dst slot, feat]) ----
            # one psum tile per 2KiB bank: psum reads are dependency-tracked
            # whole-tile, so per-bank tiles let each bank's eviction start at
            # its own stop instead of after the layer's last matmul
            psum1 = []
            for ci, (c0, cn) in enumerate(CHUNKS):
                p1t = ps.tile([128, cn], f32, tag=f"acc1_{ci}", name=f"p1_{ci}")
                psum1.append(p1t)
            for ci, (c0, cn) in enumerate(CHUNKS):
                nc.tensor.matmul(
                    psum1[ci][:, 0:cn], zl[:, 0:128], zl[:, 0:cn],
                    start=True, stop=False,
                )
            # x0 for the first four groups rides ahead of their a8 loads so
            # the DMA queue can stay a couple of groups in front of the PE
            xlead = grps[3][1] * 128
            for gi, (k0, k1) in enumerate(grps):
                a8 = a8p.tile([128, (k1 - k0) * SP], u8, tag=f"a8_{gi}")
                a8_tiles[gi] = a8
                kb = {k0, k1}
                if 3 <= gi <= 6 and (k1 - k0) % 2 == 0:
                    kb.add((k0 + k1) // 2)
                for k in range(k0, k1 + 1):
                    if k % 10 in (8, 9) and k0 < k < k1 and _rows(k - 1) != \
                            NPADROW:
                        kb.add(k)
                    if k % 10 == 0 and k0 < k < k1 and _rows(k - 1) == \
                            NPADROW:
                        kb.add(k)
                kb = sorted(kb)
                for b0, b1 in zip(kb, kb[1:]):
                    nr = NPADROW if _rows(b0) == NPADROW else 128
                    nc.sync.dma_start(
                        a8[0:nr, (b0 - k0) * SP:(b1 - k0) * SP].rearrange(
                            "p (k j) -> p k j", k=b1 - b0
                        ),
                        a[b0:b1, 0:nr].rearrange("k p j -> p k j"),
                    )
                if gi == 0:
                    nc.sync.dma_start(x0s[:, 0:xlead], x0[:, 0:xlead])
                if gi >= 4:
                    nc.sync.dma_start(
                        x0s[:, k0 * 128:k1 * 128], x0[:, k0 * 128:k1 * 128]
                    )
            ng = len(grps)
            for oi, gi in enumerate(range(ng)):
                k0, k1 = grps[gi]
                fb = cast_group(gi, deps={"halves": 2}
                                if 3 <= gi <= 6 else None)
                if oi < ng - 1:
                    for k in range(k0, k1):
                        kk = k - k0
                        nr = _rows(k)
                        rhs = x0s[0:nr, k * 128:(k + 1) * 128]
                        for t in range(10):
                            ci, tt = (t // 4, t % 4)
                            nc.tensor.matmul(
                                psum1[ci][:, tt * 128:(tt + 1) * 128],
                                fb[0:nr, kk * SP + t * 128:
                                   kk * SP + (t + 1) * 128],
                                rhs,
                                start=False, stop=False,
                            )
                else:
                    # final group t-outer: each dst range finishes early so
                    # the tanh eviction overlaps the remaining matmuls
                    for t in range(10):
                        ci, tt = (t // 4, t % 4)
                        for k in range(k0, k1):
                            kk = k - k0
                            nr = _rows(k)
                            last_mm = nc.tensor.matmul(
                                psum1[ci][:, tt * 128:(tt + 1) * 128],
                                fb[0:nr, kk * SP + t * 128:
                                   kk * SP + (t + 1) * 128],
                                x0s[0:nr, k * 128:(k + 1) * 128],
                                start=False,
                                stop=(k == k1 - 1 and t in (3, 7, 9)),
                            )

            # pre-cast layer 2's first three groups (own-rank k-tiles,
            # no x1 dependency) on DVE while layer 1 drains, so layer 2's
            # matmuls start as soon as tanh chunk 0 lands
            with tc.high_priority():
                l2fb = {gi: cast_group(gi, "v") for gi in range(3)}

            # evict layer 1: x1 = tanh(cs_dst * psum1) [* W1] on ACT; DMA to
            # the AllGather bounce per psum bank so agin lands early.  The
            # whole tanh -> agin -> AllGather -> x1s chain is the only work
            # between the two PE-bound layers, so it runs at high priority
            # and its DMAs ride the otherwise-idle SP queue.
            agin_sb = xp.tile([128, SP], f16, tag="agin")
            # keep-warm matmuls: PE would otherwise idle across the AllGather
            # valley and restart cold (2.4x slower for the first 3us)
            psumd = ps.tile([128, 512], f32, tag="warmups")
            for _ in range(ndummy):
                dmm = nc.tensor.matmul(
                    psumd[:], zl[:, 0:128], zl[:, 0:512],
                    start=True, stop=True, skip_group_check=True,
                )
                # pin behind layer 1 so the scheduler cannot hoist the
                # warm-up matmuls to the (DMA-bound) start of the program
                dmm.ins.add_sync_dependencies_from(_NameSet([last_mm.ins.name]))
            tanh_last = None
            with tc.high_priority():
                ag1 = None
                agin2_dmas = []
                for ci, (c0, cn) in enumerate(CHUNKS):
                    tanh_last = nc.scalar.activation(
                        agin_sb[:, c0:c0 + cn], psum1[ci][:, 0:cn],
                        mybir.ActivationFunctionType.Tanh,
                        scale=cscs[:, ci:ci + 1],
                    )
                    if not w1_ones:
                        nc.vector.tensor_mul(
                            agin_sb[:, c0:c0 + cn], agin_sb[:, c0:c0 + cn],
                            w1s[:, 0:cn]
                        )
                    if ci == 0:
                        ad = nc.sync.dma_start(agin1[:], agin_sb[:, 0:512])
                        if nocc:
                            ag1 = nc.sync.dma_start(
                                agout1[0:128, :], agin1[:])
                        else:
                            ag1 = nc.gpsimd.collective_compute(
                                "AllGather",
                                mybir.AluOpType.bypass,
                                replica_groups=[list(range(NCORES))],
                                ins=[agin1.opt()],
                                outs=[agout1.opt()],
                            )
                        ag1.ins.add_sync_dependencies_from(
                            _NameSet([ad.ins.name]))
                    else:
                        ad = nc.sync.dma_start(
                            agin2[:, c0 - 512:c0 - 512 + cn],
                            agin_sb[:, c0:c0 + cn])
                        agin2_dmas.append(ad.ins.name)
                if nocc:
                    ag_inst = nc.sync.dma_start(agout2[0:128, :], agin2[:])
                else:
                    ag_inst = nc.gpsimd.collective_compute(
                        "AllGather",
                        mybir.AluOpType.bypass,
                        replica_groups=[list(range(NCORES))],
                        ins=[agin2.opt()],
                        outs=[agout2.opt()],
                    )
                ag_inst.ins.add_sync_dependencies_from(
                    _NameSet(agin2_dmas))
                # A's src row-blocks are rotated per core so block 0 is the
                # core's OWN rank: layer 2's first 10 k-tiles read agin_sb
                # directly (no AllGather round-trip), and block i (i>=1) is
                # rank (pid+i)%8, fetched from agout at a register-computed
                # offset.
                ag1dep = _NameSet([ag1.ins.name])
                ag2dep = _NameSet([ag_inst.ins.name])
                pid = nc.sync.partition_id()
                for i in range(1, NCORES):
                    rank = (pid + i) % NCORES
                    dsrc = agout1[0:128, :].copy()
                    dsrc.offset = rank * (128 * 512)
                    xa = nc.sync.dma_start(
                        x1s[:, i * SP:i * SP + 512], dsrc)
                    # DRAM->SBUF reads of the collective output are not
                    # tracked as data deps; pin them behind the collectives
                    xa.ins.add_sync_dependencies_from(ag1dep)
                    dsrc = agout2[0:128, :].copy()
                    dsrc.offset = rank * (128 * (SP - 512))
                    xb = nc.sync.dma_start(
                        x1s[:, i * SP + 512:(i + 1) * SP], dsrc)
                    xb.ins.add_sync_dependencies_from(ag2dep)

            # ---- layer 2 (X-stationary; psum is [feat, dst]) ----
            # All of A is already resident as u8; only the casts re-run.
            # The first ring of casts has no x1 dependency, so it completes
            # during the AllGather and PE starts as soon as rank 0 lands.
            psum2 = []
            for ci, (c0, cn) in enumerate(CHUNKS):
                p2t = ps.tile([128, cn], f32, tag=f"acc2_{ci}", name=f"p2_{ci}")
                psum2.append(p2t)
            ob = xp.tile([128, SP], f16, tag="ob")

            def lhsT_of(k):
                # row-block 0 is the core's own rank: its activations are
                # already on-chip in agin_sb (same [dst slot, feat] layout)
                nr = _rows(k)
                if k < 10:
                    return agin_sb[0:nr, k * 128:(k + 1) * 128]
                return x1s[0:nr, k * 128:(k + 1) * 128]

            first = True
            tdep = [tanh_last.ins.name]
            adep = [ag_inst.ins.name]
            for gi, (k0, k1) in enumerate(grps):
                # keep ACT free for the tanh chain and POOL free for the
                # AllGather issue while the boundary groups pre-cast on DVE;
                # ACT/POOL rejoin once their part of the chain retires
                if gi < 3:
                    fb = l2fb[gi]
                elif gi < 5:
                    fb = cast_group(gi, "va", deps={"a": tdep})
                elif gi < 9:
                    fb = cast_group(gi, "vap", deps={"a": tdep, "p": adep})
                else:
                    fb = cast_group(gi)
                last_grp = gi == len(grps) - 1
                if not last_grp:
                    for k in range(k0, k1):
                        kk = k - k0
                        lhsT = lhsT_of(k)
                        nr = _rows(k)
                        for ci, (c0, cn) in enumerate(CH2):
                            nc.tensor.matmul(
                                psum2[ci][:, 0:cn],
                                lhsT,
                                fb[0:nr, kk * SP + c0: kk * SP + c0 + cn],
                                start=first, stop=False,
                            )
                        first = False
                else:
                    # final group: bank-outer with per-bank stops; ALL
                    # evictions are created after the matmuls (psum reads
                    # are tracked whole-tile, so an earlier-created read
                    # would falsely serialize the later banks' matmuls)
                    for ci in (2, 0, 1):
                        c0, cn = CH2[ci]
                        for k in range(k0, k1):
                            kk = k - k0
                            nc.tensor.matmul(
                                psum2[ci][:, 0:cn],
                                lhsT_of(k),
                                fb[0:_rows(k), kk * SP + c0:
                                   kk * SP + c0 + cn],
                                start=False, stop=(k == k1 - 1),
                            )
                    # GPSIMD cannot read PSUM on HW: evict banks on DVE/ACT
                    dq_eng = (nc.sync, nc.sync, nc.scalar)
                    with tc.high_priority():
                        for ci in (2, 0, 1):
                            c0, cn = CH2[ci]
                            if ci == 1:
                                nc.scalar.copy(
                                    ob[:, c0:c0 + cn], psum2[ci][:, 0:cn]
                                )
                            else:
                                nc.vector.tensor_copy(
                                    ob[:, c0:c0 + cn], psum2[ci][:, 0:cn]
                                )
                            dq_eng[ci].dma_start(
                                out[:, c0:c0 + cn], ob[:, c0:c0 + cn]
                            )

    nc.compile()
    return nc


def get_program(nocc=False, gsizes=GSIZES, ncast=NCAST, w1_ones=True,
                ndummy=NDUMMY):
    key = ("nc", nocc, tuple(gsizes), ncast, w1_ones, ndummy)
    if key not in _PROG_CACHE:
        _PROG_CACHE[key] = _build_program(nocc, gsizes, ncast, w1_ones,
                                          ndummy)
    return _PROG_CACHE[key]


def _slot_order():
    """Slot s = t*128 + p (tile t in 0..9, partition p) listed in quant-sort
    order: chunks of 4 (banks 0/1) or 2 (bank 2) consecutive sorted columns
    share one (chunk, partition) slot group, hence one dequant scale."""
    slots = np.empty(SP, np.int64)
    i = 0
    for ci, (tile0, ntile) in enumerate(((0, 4), (4, 4), (8, 2))):
        for p in range(128):
            for ti in range(ntile):
                slots[i] = (tile0 + ti) * 128 + p
                i += 1
    assert i == SP
    return slots


_SLOTS = _slot_order()


def _core_perm(colmax_ext):
    """perm[s] = original local dst column (or >=S for pad) in slot s, with
    columns sorted by quant range so slot groups share a scale."""
    order = np.argsort(-colmax_ext, kind="stable")  # [SP] sorted col ids
    perm = np.empty(SP, np.int64)
    perm[_SLOTS] = order
    return perm


def build_in_maps(x, src, dst, vals, W):
    """Host-side prep: dense A^T shard (u8 quantized, 4 sorted columns per
    scale group) + x0, both in the per-core permuted slot order."""
    import scipy.sparse as sp

    x = np.asarray(x, np.float32)
    src = np.asarray(src, np.int64)
    dst = np.asarray(dst, np.int64)
    vals = np.asarray(vals, np.float32)
    W = np.asarray(W, np.float32)

    # A[dst, src] = sum of vals  ->  we build AT[src, dst]
    AT = sp.coo_matrix((vals, (src, dst)), shape=(N, N)).toarray()

    # per-core column permutations (dst side of A, src rows of A, x rows)
    perms = []
    steps = []
    cscs = []
    for c in range(NCORES):
        ATc = AT[:, c * S:(c + 1) * S]  # [N, S] float32
        colmax_ext = np.full(SP, -1.0, np.float32)
        colmax_ext[:S] = ATc.max(axis=0)
        perm = _core_perm(colmax_ext)
        # group scale = max colmax over each slot group (same (chunk, p))
        cm_slot = np.maximum(colmax_ext[perm], 1e-9)  # [SP] by slot
        step_slot = np.empty(SP, np.float32)
        csc = np.empty((128, 3), np.float32)
        for ci, (tile0, ntile) in enumerate(((0, 4), (4, 4), (8, 2))):
            t_sl = slice(tile0 * 128, (tile0 + ntile) * 128)
            cm = cm_slot[t_sl].reshape(ntile, 128)    # [ntile, p]
            gmax = cm.max(axis=0) / 255.0             # [p]
            csc[:, ci] = gmax
            step_slot[t_sl] = np.tile(gmax[None, :], (ntile, 1)).reshape(-1)
        perms.append(perm)
        steps.append(step_slot)
        cscs.append(np.ascontiguousarray(csc))

    # per-core src slot -> node mapping: row-block i of core c is rank
    # (c+i)%8 (own rank first, so layer 2 starts from on-chip activations),
    # permuted within the block by that rank's own column permutation
    node2s, valid2s = [], []
    for c in range(NCORES):
        node2 = np.empty(NPAD, np.int64)
        valid2 = np.empty(NPAD, bool)
        for i in range(NCORES):
            r = (c + i) % NCORES
            pr = perms[r]
            valid = pr < S
            node2[i * SP:(i + 1) * SP] = np.where(valid, r * S + pr, 0)
            valid2[i * SP:(i + 1) * SP] = valid
        node2s.append(node2)
        valid2s.append(valid2)

    xw = x * W[0][None, :]

    w1brow = np.ascontiguousarray(
        np.tile(W[1][None, :], (128, 4))
    ).astype(np.float16)

    in_maps = []
    for c in range(NCORES):
        node2, valid2 = node2s[c], valid2s[c]
        x0p = np.zeros((NPAD, D), np.float32)
        x0p[valid2] = xw[node2[valid2]]
        x0h = np.ascontiguousarray(
            x0p.reshape(KT, 128, D).transpose(1, 0, 2).reshape(128, KT * D)
        ).astype(np.float16)
        ATc = AT[:, c * S:(c + 1) * S]  # [N, S] float32
        perm = perms[c]
        valid = perm < S
        ATs = np.zeros((N, SP), np.float32)
        ATs[:, valid] = ATc[:, perm[valid]]           # columns in slot order
        Aq = np.clip(np.rint(ATs / steps[c][None, :]), 0, 255).astype(
            np.uint8
        )
        Ap = np.zeros((NPAD, SP), Aq.dtype)
        Ap[valid2] = Aq[node2[valid2]]                # rows in slot order
        a3 = np.ascontiguousarray(Ap.reshape(KT, 128, SP))
        in_maps.append(
            {
                "a": a3,
                "x0": x0h,
                "csc": cscs[c],
                "w1b": w1brow,
            }
        )
    return in_maps, (steps, perms)


def assemble_output(results, aux):
    steps, perms = aux
    outs = []
    for c in range(NCORES):
        ot = np.asarray(results[c]["out"], np.float32)  # [128, SP] feat-major
        ot = ot * steps[c][None, :]  # per-dst dequant (layer-2)
        perm = perms[c]
        valid = perm < S
        o = np.zeros((S, 128), np.float32)
        o[perm[valid]] = ot[:, valid].T             # un-permute dst slots
        outs.append(o)
    return np.ascontiguousarray(np.concatenate(outs, axis=0))


def kernel(x, src, dst, vals, W):
    from concourse import bass_utils

    w1_ones = bool(np.all(np.asarray(W)[1] == 1.0))
    nc = get_program(w1_ones=w1_ones)
    in_maps, steps = build_in_maps(x, src, dst, vals, W)
    # The axon terminal can wedge when a different program was loaded
    # earlier in its lifetime; after the crash the terminal restarts and a
    # retry succeeds.  Back off progressively to ride out the restart.
    import time as _time

    last_err = None
    for sleep_s in (10.0, 30.0, 60.0, 0.0):
        try:
            res = bass_utils.run_bass_kernel_spmd(
                nc, in_maps, core_ids=list(range(NCORES))
            )
            return assemble_output(res.results, steps)
        except Exception as e:  # noqa: BLE001
            last_err = e
            _time.sleep(sleep_s)
    raise last_err


# revision 40
# speedup vs baseline: 1.0789x; 1.0024x over previous
"""GCN diag-encoder (2-layer SpMM) on 8 Trainium2 NeuronCores.

Strategy: the sparse adjacency (640K edges over 10K nodes, ~0.64% dense) is
materialized as a dense A^T matrix on the host; each per-layer
  out[dst] = sum_e vals[e] * x[src[e]]        (segment-sum SpMM)
becomes dense TensorEngine matmuls.  Each core owns a 1250-wide dst slice of
A^T (padded to 1280, uint8-quantized per dst column).

v3: A^T is DMA'd ONCE as raw uint8 (half the DMA bytes of a u8->f16
cast-DMA, which is charged at the f16 destination size) and stays resident
in SBUF (100KB/partition).  The u8->f16 conversion runs on-chip, split
across the three otherwise-idle compute engines (DVE / Activation / GpSimd)
into a rotating ring of f16 staging tiles that feed the PE.  Both layers
re-cast from the same resident u8 copy, so layer 2 needs no A traffic at
all.  This turns layer 1 from DMA-bound (~93us) into PE-bound (~45us) and
removes layer 2's 29us f16 re-stream.

Layer 1 runs A-stationary — matmul(out=psum[dst,feat], lhsT=AT_tile[src,dst],
rhs=x_tile[src,feat]) — so the layer-1 output is node-major: the eviction is
a fused tanh+dequant-scale pass on the scalar engine straight into the
AllGather bounce.  The host sorts each core's dst columns by quantization
range and packs 4 similar columns per (psum bank, partition) slot, so the
dequant scale is per-partition within a bank and the whole eviction is 3
bank-wide activations (full per-column accuracy at bank-chunk cost).  Each
psum bank is its own tile (psum reads are dependency-tracked whole-tile, so
per-bank tiles let each bank's eviction start at its own stop) and is
seeded by one full-width start=True zero matmul.  Layer 2 runs X-stationary
— matmul(out=psum[feat,dst], lhsT=x1_tile[src,feat], rhs=AT_tile[src,dst]);
its dequant scale and the final un-permute are applied on the host.

Src row-blocks are rotated per core so block 0 is the core's OWN rank:
layer 2's first 10 k-tiles read the tanh output agin_sb directly from SBUF
(no AllGather round-trip), hiding most of the collective latency behind
real work; the other 7 blocks are fetched from the AllGather output at
register-computed offsets ((partition_id+i)%8).  A few zl-by-zl keep-warm
matmuls stop the PE from dropping out of its max p-state across the
remaining gap.  W0 is folded into x on the host; W1 is skipped on device
when it is all-ones (torch init), else applied via a broadcast multiply.
"""

import numpy as np

N = 10000          # nodes
D = 128            # feature dim
NCORES = 8
S = 1250           # dst nodes per core
SP = 1280          # padded dst per core (10 tiles of 128)
KT = 80            # contraction k-tiles (padded src rows = 10240)
NPAD = KT * 128    # 10240
GSIZES = (2, 2) + (4,) * 19      # k-tiles per group (sum = 80)
NCAST = 7          # f16 staging ring depth
NDUMMY = 6         # PE keep-warm matmuls bridging the AllGather valley
# psum bank chunks: layer-1 eviction + layer-2 column blocking
CHUNKS = ((0, 512), (512, 512), (1024, 256))
# layer-2 moving spans: the last 15 dst slots (1265-1279) are always pad
# columns (the 30 pads sort last, half landing on tile 9's tail), so the
# layer-2 matmuls skip them outright
CH2 = ((0, 512), (512, 512), (1024, 226))
# the 30 pad columns per core sort to partitions 113..127 of tiles 8 and 9,
# so k-tiles with k%10 in (8,9) only have 113 real src rows
NPADROW = 98


def _rows(k):
    return NPADROW if k % 10 == 9 else 128

_PROG_CACHE = {}


def _build_program(nocc=False, gsizes=GSIZES, ncast=NCAST, w1_ones=True,
                   ndummy=NDUMMY):
    import concourse.bacc as bacc
    import concourse.mybir as mybir
    from bass_rust import InstructionNameOrderedSet as _NameSet
    from concourse import tile

    f32 = mybir.dt.float32
    f16 = mybir.dt.float16
    u8 = mybir.dt.uint8
    grps = []
    _k0 = 0
    for _sz in gsizes:
        grps.append((_k0, _k0 + _sz))
        _k0 += _sz
    assert _k0 == KT
    maxg = max(k1 - k0 for k0, k1 in grps)

    nc = bacc.Bacc(
        "TRN2",
        target_bir_lowering=False,
        debug=False,
        enable_asserts=False,
        num_devices=1 if nocc else NCORES,
    )

    a = nc.dram_tensor("a", [KT, 128, SP], u8, kind="ExternalInput").ap()
    x0 = nc.dram_tensor("x0", [128, NPAD], f16, kind="ExternalInput").ap()
    # per-(bank chunk, partition) dequant scales; the host sorts dst
    # columns by quant range so each (chunk, partition) slot's 4 columns
    # share one scale -> the tanh eviction is 3 bank-wide activations
    csc = nc.dram_tensor("csc", [128, 3], f32, kind="ExternalInput").ap()
    # broadcast W1 row tiled x4 (only read when not w1_ones)
    w1b = nc.dram_tensor("w1b", [128, 512], f16, kind="ExternalInput").ap()
    out = nc.dram_tensor("out", [128, SP], f16, kind="ExternalOutput").ap()

    with tile.TileContext(nc) as tc:
        with (
            tc.tile_pool(name="xp", bufs=1) as xp,
            tc.tile_pool(name="a8p", bufs=1) as a8p,
            tc.tile_pool(name="fc", bufs=ncast) as fcp,
            tc.tile_pool(name="ps", bufs=1, space="PSUM") as ps,
            tc.tile_pool(name="dr", bufs=1, space="DRAM") as dr,
        ):
            # x0 is dead once layer 1 finishes; share one slot for both
            x0s = xp.tile([128, NPAD], f16, tag="xs")
            x1s = xp.tile([128, NPAD], f16, tag="xs")
            cscs = xp.tile([128, 3], f32, tag="cscs")
            w1s = xp.tile([128, 512], f16, tag="w1s")
            zl = xp.tile([128, 512], f16, tag="zl")
            warm = xp.tile([128, 1], f32, tag="warm")
            nc.scalar.dma_start(cscs[:], csc)
            if not w1_ones:
                nc.scalar.dma_start(w1s[:], w1b)
            nc.vector.memset(zl[:, 0:128], 0.0)
            nc.vector.memset(zl[:, 128:512], 0.0)
            # pre-load the ACT tanh table so the layer-1 eviction doesn't
            # pay the table load on the critical path
            nc.scalar.activation(
                warm[:], zl[:, 0:1], mybir.ActivationFunctionType.Tanh
            )

            # the AllGather is split in two chunked collectives so the
            # first remote block's head (cols 0-512) lands right after tanh
            # chunk 0, shrinking the boundary valley
            agin1 = dr.tile([128, 512], f16)
            agin2 = dr.tile([128, SP - 512], f16)
            agout1 = dr.tile([NCORES * 128, 512], f16, addr_space="Shared")
            agout2 = dr.tile([NCORES * 128, SP - 512], f16,
                             addr_space="Shared")

            a8_tiles = {}

            def cast_group(gi, engines="vap", deps=None):
                """u8 -> f16 of resident group gi, split over the engines in
                `engines` (v=DVE, a=ACT in ~1us slices, p=POOL), shares
                proportional to their elementwise rates.  `deps` maps an
                engine letter to instruction names the slice must follow —
                used at the layer boundary so the scheduler cannot hoist
                casts ahead of the tanh -> AllGather chain."""
                k0, k1 = grps[gi]
                w = (k1 - k0) * SP
                a8 = a8_tiles[gi]
                fb = fcp.tile([128, maxg * SP], f16, tag="fc")
                rates = {"v": 4, "a": 4, "p": 3}
                tot = sum(rates[e] for e in engines)
                halves = deps.pop("halves", 1) if deps else 1
                bounds = [(w * h) // halves for h in range(halves + 1)]
                for h0, h1 in zip(bounds, bounds[1:]):
                  wh = h1 - h0
                  c0 = h0
                  for e in engines:
                    c1 = h1 if e == engines[-1] else c0 + (wh * rates[e]) // tot
                    insts = []
                    if e == "v":
                        insts.append(
                            nc.vector.tensor_copy(fb[:, c0:c1], a8[:, c0:c1])
                        )
                    elif e == "a":
                        # <=1.3k-elem slices so ACT never blocks the layer-1
                        # eviction chain behind a long copy
                        s0 = c0
                        while s0 < c1:
                            s1 = min(s0 + 1280, c1)
                            insts.append(
                                nc.scalar.copy(fb[:, s0:s1], a8[:, s0:s1])
                            )
                            s0 = s1
                    else:
                        insts.append(
                            nc.gpsimd.tensor_copy(fb[:, c0:c1], a8[:, c0:c1])
                        )
                    if deps and e in deps:
                        for inst in insts:
                            inst.ins.add_sync_dependencies_from(_NameSet(deps[e]))
                    c0 = c1
                return fb

            # ---- layer 1 (A-stationary; psum is [dst slot, feat]) ----
            # one psum tile per 2KiB bank: psum reads are dependency-tracked
            # whole-tile, so per-bank tiles let each bank's eviction start at
            # its own stop instead of after the layer's last matmul
            psum1 = []
            for ci, (c0, cn) in enumerate(CHUNKS):
                p1t = ps.tile([128, cn], f32, tag=f"acc1_{ci}", name=f"p1_{ci}")
                psum1.append(p1t)
            for ci, (c0, cn) in enumerate(CHUNKS):
                nc.tensor.matmul(
                    psum1[ci][:, 0:cn], zl[:, 0:128], zl[:, 0:cn],
                    start=True, stop=False,
                )
            # x0 for the first four groups rides ahead of their a8 loads so
            # the DMA queue can stay a couple of groups in front of the PE
            xlead = grps[3][1] * 128
            for gi, (k0, k1) in enumerate(grps):
                a8 = a8p.tile([128, (k1 - k0) * SP], u8, tag=f"a8_{gi}")
                a8_tiles[gi] = a8
                kb = {k0, k1}
                if 3 <= gi <= 6 and (k1 - k0) % 2 == 0:
                    kb.add((k0 + k1) // 2)
                for k in range(k0 + 1, k1):
                    if _rows(k) != _rows(k - 1):
                        kb.add(k)
                kb = sorted(kb)
                for b0, b1 in zip(kb, kb[1:]):
                    nr = _rows(b0)
                    nc.sync.dma_start(
                        a8[0:nr, (b0 - k0) * SP:(b1 - k0) * SP].rearrange(
                            "p (k j) -> p k j", k=b1 - b0
                        ),
                        a[b0:b1, 0:nr].rearrange("k p j -> p k j"),
                    )
                if gi == 0:
                    nc.sync.dma_start(x0s[:, 0:xlead], x0[:, 0:xlead])
                if gi >= 4:
                    nc.sync.dma_start(
                        x0s[:, k0 * 128:k1 * 128], x0[:, k0 * 128:k1 * 128]
                    )
            ng = len(grps)
            for oi, gi in enumerate(range(ng)):
                k0, k1 = grps[gi]
                fb = cast_group(gi, deps={"halves": 2}
                                if 3 <= gi <= 6 else None)
                if oi < ng - 1:
                    for k in range(k0, k1):
                        kk = k - k0
                        nr = _rows(k)
                        rhs = x0s[0:nr, k * 128:(k + 1) * 128]
                        for t in range(10):
                            ci, tt = (t // 4, t % 4)
                            nc.tensor.matmul(
                                psum1[ci][:, tt * 128:(tt + 1) * 128],
                                fb[0:nr, kk * SP + t * 128:
                                   kk * SP + (t + 1) * 128],
                                rhs,
                                start=False, stop=False,
                            )
                else:
                    # final group t-outer: each dst range finishes early so
                    # the tanh eviction overlaps the remaining matmuls
                    for t in range(10):
                        ci, tt = (t // 4, t % 4)
                        for k in range(k0, k1):
                            kk = k - k0
                            nr = _rows(k)
                            last_mm = nc.tensor.matmul(
                                psum1[ci][:, tt * 128:(tt + 1) * 128],
                                fb[0:nr, kk * SP + t * 128:
                                   kk * SP + (t + 1) * 128],
                                x0s[0:nr, k * 128:(k + 1) * 128],
                                start=False,
                                stop=(k == k1 - 1 and t in (3, 7, 9)),
                            )

            # pre-cast layer 2's first three groups (own-rank k-tiles,
            # no x1 dependency) on DVE while layer 1 drains, so layer 2's
            # matmuls start as soon as tanh chunk 0 lands
            with tc.high_priority():
                l2fb = {gi: cast_group(gi, "v") for gi in range(3)}

            # evict layer 1: x1 = tanh(cs_dst * psum1) [* W1] on ACT; DMA to
            # the AllGather bounce per psum bank so agin lands early.  The
            # whole tanh -> agin -> AllGather -> x1s chain is the only work
            # between the two PE-bound layers, so it runs at high priority
            # and its DMAs ride the otherwise-idle SP queue.
            agin_sb = xp.tile([128, SP], f16, tag="agin")
            # keep-warm matmuls: PE would otherwise idle across the AllGather
            # valley and restart cold (2.4x slower for the first 3us)
            psumd = ps.tile([128, 512], f32, tag="warmups")
            for _ in range(ndummy):
                dmm = nc.tensor.matmul(
                    psumd[:], zl[:, 0:128], zl[:, 0:512],
                    start=True, stop=True, skip_group_check=True,
                )
                # pin behind layer 1 so the scheduler cannot hoist the
                # warm-up matmuls to the (DMA-bound) start of the program
                dmm.ins.add_sync_dependencies_from(_NameSet([last_mm.ins.name]))
            tanh_last = None
            with tc.high_priority():
                ag1 = None
                agin2_dmas = []
                for ci, (c0, cn) in enumerate(CHUNKS):
                    tanh_last = nc.scalar.activation(
                        agin_sb[:, c0:c0 + cn], psum1[ci][:, 0:cn],
                        mybir.ActivationFunctionType.Tanh,
                        scale=cscs[:, ci:ci + 1],
                    )
                    if not w1_ones:
                        nc.vector.tensor_mul(
                            agin_sb[:, c0:c0 + cn], agin_sb[:, c0:c0 + cn],
                            w1s[:, 0:cn]
                        )
                    if ci == 0:
                        ad = nc.sync.dma_start(agin1[:], agin_sb[:, 0:512])
                        if nocc:
                            ag1 = nc.sync.dma_start(
                                agout1[0:128, :], agin1[:])
                        else:
                            ag1 = nc.gpsimd.collective_compute(
                                "AllGather",
                                mybir.AluOpType.bypass,
                                replica_groups=[list(range(NCORES))],
                                ins=[agin1.opt()],
                                outs=[agout1.opt()],
                            )
                        ag1.ins.add_sync_dependencies_from(
                            _NameSet([ad.ins.name]))
                    else:
                        ad = nc.sync.dma_start(
                            agin2[:, c0 - 512:c0 - 512 + cn],
                            agin_sb[:, c0:c0 + cn])
                        agin2_dmas.append(ad.ins.name)
                if nocc:
                    ag_inst = nc.sync.dma_start(agout2[0:128, :], agin2[:])
                else:
                    ag_inst = nc.gpsimd.collective_compute(
                        "AllGather",
                        mybir.AluOpType.bypass,
                        replica_groups=[list(range(NCORES))],
                        ins=[agin2.opt()],
                        outs=[agout2.opt()],
                    )
                ag_inst.ins.add_sync_dependencies_from(
                    _NameSet(agin2_dmas))
                # A's src row-blocks are rotated per core so block 0 is the
                # core's OWN rank: layer 2's first 10 k-tiles read agin_sb
                # directly (no AllGather round-trip), and block i (i>=1) is
                # rank (pid+i)%8, fetched from agout at a register-computed
                # offset.
                ag1dep = _NameSet([ag1.ins.name])
                ag2dep = _NameSet([ag_inst.ins.name])
                pid = nc.sync.partition_id()
                for i in range(1, NCORES):
                    rank = (pid + i) % NCORES
                    dsrc = agout1[0:128, :].copy()
                    dsrc.offset = rank * (128 * 512)
                    xa = nc.sync.dma_start(
                        x1s[:, i * SP:i * SP + 512], dsrc)
                    # DRAM->SBUF reads of the collective output are not
                    # tracked as data deps; pin them behind the collectives
                    xa.ins.add_sync_dependencies_from(ag1dep)
                    dsrc = agout2[0:128, :].copy()
                    dsrc.offset = rank * (128 * (SP - 512))
                    xb = nc.sync.dma_start(
                        x1s[:, i * SP + 512:(i + 1) * SP], dsrc)
                    xb.ins.add_sync_dependencies_from(ag2dep)

            # ---- layer 2 (X-stationary; psum is [feat, dst]) ----
            # All of A is already resident as u8; only the casts re-run.
            # The first ring of casts has no x1 dependency, so it completes
            # during the AllGather and PE starts as soon as rank 0 lands.
            psum2 = []
            for ci, (c0, cn) in enumerate(CHUNKS):
                p2t = ps.tile([128, cn], f32, tag=f"acc2_{ci}", name=f"p2_{ci}")
                psum2.append(p2t)
            ob = xp.tile([128, SP], f16, tag="ob")

            def lhsT_of(k):
                # row-block 0 is the core's own rank: its activations are
                # already on-chip in agin_sb (same [dst slot, feat] layout)
                nr = _rows(k)
                if k < 10:
                    return agin_sb[0:nr, k * 128:(k + 1) * 128]
                return x1s[0:nr, k * 128:(k + 1) * 128]

            first = True
            tdep = [tanh_last.ins.name]
            adep = [ag_inst.ins.name]
            for gi, (k0, k1) in enumerate(grps):
                # keep ACT free for the tanh chain and POOL free for the
                # AllGather issue while the boundary groups pre-cast on DVE;
                # ACT/POOL rejoin once their part of the chain retires
                if gi < 3:
                    fb = l2fb[gi]
                elif gi < 5:
                    fb = cast_group(gi, "va", deps={"a": tdep})
                elif gi < 9:
                    fb = cast_group(gi, "vap", deps={"a": tdep, "p": adep})
                else:
                    fb = cast_group(gi)
                last_grp = gi == len(grps) - 1
                if not last_grp:
                    for k in range(k0, k1):
                        kk = k - k0
                        lhsT = lhsT_of(k)
                        nr = _rows(k)
                        for ci, (c0, cn) in enumerate(CH2):
                            nc.tensor.matmul(
                                psum2[ci][:, 0:cn],
                                lhsT,
                                fb[0:nr, kk * SP + c0: kk * SP + c0 + cn],
                                start=first, stop=False,
                            )
                        first = False
                else:
                    # final group: bank-outer with per-bank stops; ALL
                    # evictions are created after the matmuls (psum reads
                    # are tracked whole-tile, so an earlier-created read
                    # would falsely serialize the later banks' matmuls)
                    for ci in (2, 0, 1):
                        c0, cn = CH2[ci]
                        for k in range(k0, k1):
                            kk = k - k0
                            nc.tensor.matmul(
                                psum2[ci][:, 0:cn],
                                lhsT_of(k),
                                fb[0:_rows(k), kk * SP + c0:
                                   kk * SP + c0 + cn],
                                start=False, stop=(k == k1 - 1),
                            )
                    # GPSIMD cannot read PSUM on HW: evict banks on DVE/ACT
                    dq_eng = (nc.sync, nc.sync, nc.scalar)
                    with tc.high_priority():
                        for ci in (2, 0, 1):
                            c0, cn = CH2[ci]
                            if ci == 1:
                                nc.scalar.copy(
                                    ob[:, c0:c0 + cn], psum2[ci][:, 0:cn]
                                )
                            else:
                                nc.vector.tensor_copy(
                                    ob[:, c0:c0 + cn], psum2[ci][:, 0:cn]
                                )
                            dq_eng[ci].dma_start(
                                out[:, c0:c0 + cn], ob[:, c0:c0 + cn]
                            )

    nc.compile()
    return nc


def get_program(nocc=False, gsizes=GSIZES, ncast=NCAST, w1_ones=True,
                ndummy=NDUMMY):
    key = ("nc", nocc, tuple(gsizes), ncast, w1_ones, ndummy)
    if key not in _PROG_CACHE:
        _PROG_CACHE[key] = _build_program(nocc, gsizes, ncast, w1_ones,
                                          ndummy)
    return _PROG_CACHE[key]


def _slot_order():
    """Slot s = t*128 + p (tile t in 0..9, partition p) listed in quant-sort
    order: chunks of 4 (banks 0/1) or 2 (bank 2) consecutive sorted columns
    share one (chunk, partition) slot group, hence one dequant scale."""
    slots = np.empty(SP, np.int64)
    i = 0
    for ci, (tile0, ntile) in enumerate(((0, 4), (4, 4), (8, 2))):
        if ci < 2:
            for p in range(128):
                for ti in range(ntile):
                    slots[i] = (tile0 + ti) * 128 + p
                    i += 1
        else:
            # tile-outer so the 30 pad columns (sorted last) become the
            # contiguous tail of tile 9 (slots 1250-1279)
            for ti in range(ntile):
                for p in range(128):
                    slots[i] = (tile0 + ti) * 128 + p
                    i += 1
    assert i == SP
    return slots


_SLOTS = _slot_order()


def _core_perm(colmax_ext):
    """perm[s] = original local dst column (or >=S for pad) in slot s, with
    columns sorted by quant range so slot groups share a scale."""
    order = np.argsort(-colmax_ext, kind="stable")  # [SP] sorted col ids
    perm = np.empty(SP, np.int64)
    perm[_SLOTS] = order
    return perm


def build_in_maps(x, src, dst, vals, W):
    """Host-side prep: dense A^T shard (u8 quantized, 4 sorted columns per
    scale group) + x0, both in the per-core permuted slot order."""
    import scipy.sparse as sp

    x = np.asarray(x, np.float32)
    src = np.asarray(src, np.int64)
    dst = np.asarray(dst, np.int64)
    vals = np.asarray(vals, np.float32)
    W = np.asarray(W, np.float32)

    # A[dst, src] = sum of vals  ->  we build AT[src, dst]
    AT = sp.coo_matrix((vals, (src, dst)), shape=(N, N)).toarray()

    # per-core column permutations (dst side of A, src rows of A, x rows)
    perms = []
    steps = []
    cscs = []
    for c in range(NCORES):
        ATc = AT[:, c * S:(c + 1) * S]  # [N, S] float32
        colmax_ext = np.full(SP, -1.0, np.float32)
        colmax_ext[:S] = ATc.max(axis=0)
        perm = _core_perm(colmax_ext)
        # group scale = max colmax over each slot group (same (chunk, p))
        cm_slot = np.maximum(colmax_ext[perm], 1e-9)  # [SP] by slot
        step_slot = np.empty(SP, np.float32)
        csc = np.empty((128, 3), np.float32)
        for ci, (tile0, ntile) in enumerate(((0, 4), (4, 4), (8, 2))):
            t_sl = slice(tile0 * 128, (tile0 + ntile) * 128)
            cm = cm_slot[t_sl].reshape(ntile, 128)    # [ntile, p]
            gmax = cm.max(axis=0) / 255.0             # [p]
            csc[:, ci] = gmax
            step_slot[t_sl] = np.tile(gmax[None, :], (ntile, 1)).reshape(-1)
        perms.append(perm)
        steps.append(step_slot)
        cscs.append(np.ascontiguousarray(csc))

    # per-core src slot -> node mapping: row-block i of core c is rank
    # (c+i)%8 (own rank first, so layer 2 starts from on-chip activations),
    # permuted within the block by that rank's own column permutation
    node2s, valid2s = [], []
    for c in range(NCORES):
        node2 = np.empty(NPAD, np.int64)
        valid2 = np.empty(NPAD, bool)
        for i in range(NCORES):
            r = (c + i) % NCORES
            pr = perms[r]
            valid = pr < S
            node2[i * SP:(i + 1) * SP] = np.where(valid, r * S + pr, 0)
            valid2[i * SP:(i + 1) * SP] = valid
        node2s.append(node2)
        valid2s.append(valid2)

    xw = x * W[0][None, :]

    w1brow = np.ascontiguousarray(
        np.tile(W[1][None, :], (128, 4))
    ).astype(np.float16)

    in_maps = []
    for c in range(NCORES):
        node2, valid2 = node2s[c], valid2s[c]
        x0p = np.zeros((NPAD, D), np.float32)
        x0p[valid2] = xw[node2[valid2]]
        x0h = np.ascontiguousarray(
            x0p.reshape(KT, 128, D).transpose(1, 0, 2).reshape(128, KT * D)
        ).astype(np.float16)
        ATc = AT[:, c * S:(c + 1) * S]  # [N, S] float32
        perm = perms[c]
        valid = perm < S
        ATs = np.zeros((N, SP), np.float32)
        ATs[:, valid] = ATc[:, perm[valid]]           # columns in slot order
        Aq = np.clip(np.rint(ATs / steps[c][None, :]), 0, 255).astype(
            np.uint8
        )
        Ap = np.zeros((NPAD, SP), Aq.dtype)
        Ap[valid2] = Aq[node2[valid2]]                # rows in slot order
        a3 = np.ascontiguousarray(Ap.reshape(KT, 128, SP))
        in_maps.append(
            {
                "a": a3,
                "x0": x0h,
                "csc": cscs[c],
                "w1b": w1brow,
            }
        )
    return in_maps, (steps, perms)


def assemble_output(results, aux):
    steps, perms = aux
    outs = []
    for c in range(NCORES):
        ot = np.asarray(results[c]["out"], np.float32)  # [128, SP] feat-major
        ot = ot * steps[c][None, :]  # per-dst dequant (layer-2)
        perm = perms[c]
        valid = perm < S
        o = np.zeros((S, 128), np.float32)
        o[perm[valid]] = ot[:, valid].T             # un-permute dst slots
        outs.append(o)
    return np.ascontiguousarray(np.concatenate(outs, axis=0))


def kernel(x, src, dst, vals, W):
    from concourse import bass_utils

    w1_ones = bool(np.all(np.asarray(W)[1] == 1.0))
    nc = get_program(w1_ones=w1_ones)
    in_maps, steps = build_in_maps(x, src, dst, vals, W)
    # The axon terminal can wedge when a different program was loaded
    # earlier in its lifetime; after the crash the terminal restarts and a
    # retry succeeds.  Back off progressively to ride out the restart.
    import time as _time

    last_err = None
    for sleep_s in (10.0, 30.0, 60.0, 0.0):
        try:
            res = bass_utils.run_bass_kernel_spmd(
                nc, in_maps, core_ids=list(range(NCORES))
            )
            return assemble_output(res.results, steps)
        except Exception as e:  # noqa: BLE001
            last_err = e
            _time.sleep(sleep_s)
    raise last_err
